# revision 1
# baseline (speedup 1.0000x reference)
"""Trainium2 Bass kernel for nn_DCNConvModule (modulated deformable conv
+ GroupNorm(1) + ReLU).

Sharding: 8 cores; core (2b + h) computes sample b, output rows [32h, 32h+32).
GroupNorm statistics are per-sample -> tiny AllReduce of (sum, sumsq) within
core pairs [[0,1],[2,3],[4,5],[6,7]].

Per-core algorithm (pixel-major "px" = 2048 output pixels on 16 tiles of 128):
  1. offset conv as 9-tap PE matmuls -> off [27, px] -> PE transpose -> [px, 27]
  2. coef/index math on DVE; a zero-padded transposed gather table in DRAM
     (66-wide grid, +1 coordinate offset) makes all bilinear corner validity
     masking implicit.
  3. per tap: dma_gather fetches 2-row pairs (4 bilinear corners in 2 calls),
     4 scalar_tensor_tensor FMAs combine corners with per-partition coefs,
     PE-transpose of val, 4 matmuls accumulate y[256, 1024] in PSUM.
  4. GN: per-partition sums via ACT accumulators, ones-matmul partition
     reduce, pair AllReduce, normalize+ReLU as one ACT op per chunk.
"""
import contextlib
import numpy as np

K = 3
KK = 9
C = 256
CO = 256
H = 64
W = 64
B = 4
GW = 66                  # padded grid width
TBL_ROWS = 68 * GW + 8   # 4496: grid rows 0..67 (+ slack)
NWIN = TBL_ROWS - 2      # overlapping 512-elem windows, stride 256
PXT = 16                 # 128-pixel tiles per core
NPX = PXT * 128          # 2048 pixels per core
GN_EPS = 1e-5
NCORES = 8

_cache = {}


# ----------------------------------------------------------------- host prep
def prep_per_core(x, w_off, b_off, w, b, gamma, beta):
    """Build the 8 per-core input maps (all numpy, layout-only work)."""
    ki = np.arange(KK) // K
    kj = np.arange(KK) % K

    # conv_offset lhsT  [128, 2, 9, 27]: [c', cc, tap, o]
    woff_r = np.ascontiguousarray(
        w_off.reshape(27, 2, 128, K, K).transpose(2, 1, 3, 4, 0)
        .reshape(128, 2, 9, 27)).astype(np.float32)
    # main DCN lhsT  [128, 9, 2, 2, 128]: [c', k, cc, oh, o']
    w2 = np.ascontiguousarray(
        w.reshape(2, 128, 2, 128, KK).transpose(3, 4, 2, 0, 1)
    ).astype(np.float32)
    boff = b_off.reshape(27, 1).astype(np.float32).copy()
    bvec = np.ascontiguousarray(b.reshape(2, 128).T).astype(np.float32)
    gam2 = np.ascontiguousarray(gamma.reshape(2, 128).T).astype(np.float32)
    bet2 = np.ascontiguousarray(beta.reshape(2, 128).T).astype(np.float32)

    p = np.arange(128)
    basex = (p[:, None] % 64 + kj[None, :]).astype(np.float32)  # [128, 9]

    x_pad = np.zeros((B, C, H + 2, W + 2), np.float32)
    x_pad[:, :, 1:H+1, 1:W+1] = x

    in_maps = []
    for core in range(NCORES):
        bi, hh = core // 2, core % 2
        h0 = hh * 32
        tbl = np.zeros((TBL_ROWS, C), np.float32)
        grid = np.zeros((68, GW, C), np.float32)
        grid[:66, :, :] = x_pad[bi].transpose(1, 2, 0)
        tbl[:68 * GW] = grid.reshape(68 * GW, C)
        slab = x_pad[bi][:, h0:h0+34, :]                    # [256, 34, 66]
        xc = np.ascontiguousarray(
            slab.reshape(2, 128, 34 * 66).transpose(1, 0, 2))
        t = np.arange(PXT)
        basey = np.ascontiguousarray(
            (h0 + (t[None, :, None] * 128 + p[:, None, None]) // 64
             + ki[None, None, :])).astype(np.float32)
        rep = np.zeros((16, 128), np.float32)
        rep[np.arange(128) % 16, np.arange(128)] = 1.0
        in_maps.append(dict(
            xt=np.ascontiguousarray(tbl), xc=xc,
            woff=woff_r, boff=boff, w2=w2,
            bvec=bvec, gam2=gam2, bet2=bet2,
            basey=basey, basex=basex, rep16=rep,
        ))
    return in_maps


# --------------------------------------------------------------- bass kernel
def build_module(use_collective=True):
    import concourse.bacc as bacc
    import concourse.bass as bass
    import concourse.tile as tile
    from concourse import mybir
    from concourse.masks import make_identity

    f32 = mybir.dt.float32
    i32 = mybir.dt.int32
    i16 = mybir.dt.int16
    Alu = mybir.AluOpType
    Act = mybir.ActivationFunctionType

    nc = bacc.Bacc("TRN2", target_bir_lowering=False, debug=False,
                   num_devices=NCORES)

    xt = nc.dram_tensor("xt", [TBL_ROWS, C], f32, kind="ExternalInput")
    xc = nc.dram_tensor("xc", [128, 2, 34 * 66], f32, kind="ExternalInput")
    woff = nc.dram_tensor("woff", [128, 2, 9, 27], f32, kind="ExternalInput")
    boff = nc.dram_tensor("boff", [27, 1], f32, kind="ExternalInput")
    w2 = nc.dram_tensor("w2", [128, 9, 2, 2, 128], f32, kind="ExternalInput")
    bvec = nc.dram_tensor("bvec", [128, 2], f32, kind="ExternalInput")
    gam2 = nc.dram_tensor("gam2", [128, 2], f32, kind="ExternalInput")
    bet2 = nc.dram_tensor("bet2", [128, 2], f32, kind="ExternalInput")
    basey = nc.dram_tensor("basey", [128, PXT, 9], f32, kind="ExternalInput")
    basex = nc.dram_tensor("basex", [128, 9], f32, kind="ExternalInput")
    rep16 = nc.dram_tensor("rep16", [16, 128], f32, kind="ExternalInput")
    yout = nc.dram_tensor("yout", [CO, NPX], f32, kind="ExternalOutput")

    cc_in = nc.dram_tensor("cc_in", [1, 8], f32)
    cc_out = nc.dram_tensor("cc_out", [1, 8], f32)

    # gather source: overlapping 512-elem windows with 256-elem stride
    xt_win = bass.AP(tensor=xt, offset=0, ap=[[C, NWIN], [1, C * 2]])

    def swap_free(ap2):
        """Swap the two free dims of a [P, A, B] AP (iteration order only)."""
        return bass.AP(tensor=ap2.tensor, offset=ap2.offset,
                       ap=[ap2.ap[0], ap2.ap[2], ap2.ap[1]])

    with tile.TileContext(nc) as tc, contextlib.ExitStack() as ctx:
        consts = ctx.enter_context(tc.tile_pool(name="consts", bufs=1))
        sb = ctx.enter_context(tc.tile_pool(name="sb", bufs=1))
        ps_small = ctx.enter_context(
            tc.tile_pool(name="ps_small", bufs=1, space="PSUM"))
        gat = ctx.enter_context(tc.tile_pool(name="gat", bufs=2))
        vals = ctx.enter_context(tc.tile_pool(name="vals", bufs=6))
        ps_vt = ctx.enter_context(
            tc.tile_pool(name="ps_vt", bufs=3, space="PSUM"))
        ps_y = ctx.enter_context(
            tc.tile_pool(name="ps_y", bufs=1, space="PSUM"))

        ident = consts.tile([128, 128], f32)
        make_identity(nc, ident[:])
        ones_row = consts.tile([1, 128], f32)
        nc.vector.memset(ones_row[:], 1.0)
        ones_col = consts.tile([128, 1], f32)
        nc.vector.memset(ones_col[:], 1.0)
        eps_t = consts.tile([1, 1], f32)
        nc.vector.memset(eps_t[:], GN_EPS)

        xc_sb = consts.tile([128, 2, 34 * 66], f32)
        nc.sync.dma_start(out=xc_sb[:], in_=xc[:])
        woff_sb = consts.tile([128, 2, 9, 27], f32)
        nc.sync.dma_start(out=woff_sb[:], in_=woff[:])
        w2_sb = consts.tile([128, 9, 2, 2, 128], f32)
        nc.sync.dma_start(out=w2_sb[:], in_=w2[:])
        boff_sb = consts.tile([27, 1], f32)
        nc.sync.dma_start(out=boff_sb[:], in_=boff[:])
        bvec_sb = consts.tile([128, 2], f32)
        nc.sync.dma_start(out=bvec_sb[:], in_=bvec[:])
        gam_sb = consts.tile([128, 2], f32)
        nc.sync.dma_start(out=gam_sb[:], in_=gam2[:])
        bet_sb = consts.tile([128, 2], f32)
        nc.sync.dma_start(out=bet_sb[:], in_=bet2[:])
        basey_sb = consts.tile([128, PXT, 9], f32)
        nc.sync.dma_start(out=basey_sb[:], in_=basey[:])
        basex_sb = consts.tile([128, 9], f32)
        nc.sync.dma_start(out=basex_sb[:], in_=basex[:])
        rep16_sb = consts.tile([16, 128], f32)
        nc.sync.dma_start(out=rep16_sb[:], in_=rep16[:])

        # ---------------- phase 1: offset conv  off[27, px] ----------------
        off_sb = sb.tile([27, 4, 512], f32)
        xcv = [xc_sb[:, cc, :].rearrange("p (r c) -> p r c", c=66)
               for cc in range(2)]
        for ch in range(4):
            ps_off = ps_small.tile([27, 512], f32, tag="ps")
            n = 0
            for cc in range(2):
                for a in range(3):
                    for bb in range(3):
                        nc.tensor.matmul(
                            ps_off[:, :],
                            woff_sb[:, cc, a * 3 + bb, :],
                            xcv[cc][:, ch * 8 + a: ch * 8 + a + 8, bb: bb + 64],
                            start=(n == 0), stop=(n == 17))
                        n += 1
            nc.vector.tensor_scalar_add(out=off_sb[:, ch, :], in0=ps_off[:, :],
                                        scalar1=boff_sb[:, 0:1])

        # transpose -> offT [px 128, 16, 27]
        offT = sb.tile([128, PXT, 27], f32)
        for t in range(PXT):
            ch, sub = t // 4, t % 4
            ps_t = ps_small.tile([128, 27], f32, tag="ps")
            nc.tensor.transpose(
                ps_t[:, :],
                off_sb[:, ch, sub * 128:(sub + 1) * 128],
                ident[0:27, 0:27])
            nc.scalar.copy(out=offT[:, t, :], in_=ps_t[:, :])

        # ---------------- phase 2: coef + index math (batched) -------------
        dy = offT[:, :, 0:18:2]     # [128, 16, 9] strided views
        dx = offT[:, :, 1:18:2]
        moff = offT[:, :, 18:27]

        msk = sb.tile([128, PXT, 9], f32)
        nc.scalar.activation(out=msk[:], in_=moff, func=Act.Sigmoid)

        pyg = sb.tile([128, PXT, 9], f32)
        nc.vector.tensor_tensor(out=pyg[:], in0=dy, in1=basey_sb[:], op=Alu.add)
        pxg = sb.tile([128, PXT, 9], f32)
        bxa = basex_sb[:]
        bx_b = bass.AP(tensor=bxa.tensor, offset=bxa.offset,
                       ap=[bxa.ap[0], [0, PXT], [1, 9]])
        nc.vector.tensor_tensor(out=pxg[:], in0=dx, in1=bx_b, op=Alu.add)

        def floor_(src, dst_f, dst_frac, dst_clamp):
            ii = sb.tile([128, PXT, 9], i32, tag="flo_i")
            nc.vector.tensor_copy(out=ii[:], in_=src[:])
            ff = sb.tile([128, PXT, 9], f32, tag="flo_f")
            nc.vector.tensor_copy(out=ff[:], in_=ii[:])
            gt = sb.tile([128, PXT, 9], f32, tag="flo_g")
            nc.vector.tensor_tensor(out=gt[:], in0=ff[:], in1=src[:],
                                    op=Alu.is_gt)
            nc.vector.tensor_tensor(out=dst_f[:], in0=ff[:], in1=gt[:],
                                    op=Alu.subtract)
            nc.vector.tensor_tensor(out=dst_frac[:], in0=src[:], in1=dst_f[:],
                                    op=Alu.subtract)
            nc.vector.tensor_scalar(out=dst_clamp[:], in0=dst_f[:],
                                    scalar1=0.0, scalar2=65.0,
                                    op0=Alu.max, op1=Alu.min)

        y0f = sb.tile([128, PXT, 9], f32)
        ly = sb.tile([128, PXT, 9], f32)
        y0c = sb.tile([128, PXT, 9], f32)
        floor_(pyg, y0f, ly, y0c)
        x0f = sb.tile([128, PXT, 9], f32)
        lx = sb.tile([128, PXT, 9], f32)
        x0c = sb.tile([128, PXT, 9], f32)
        floor_(pxg, x0f, lx, x0c)

        ly1 = sb.tile([128, PXT, 9], f32)
        nc.vector.tensor_scalar(out=ly1[:], in0=ly[:], scalar1=-1.0,
                                scalar2=1.0, op0=Alu.mult, op1=Alu.add)
        lx1 = sb.tile([128, PXT, 9], f32)
        nc.vector.tensor_scalar(out=lx1[:], in0=lx[:], scalar1=-1.0,
                                scalar2=1.0, op0=Alu.mult, op1=Alu.add)
        ay0 = sb.tile([128, PXT, 9], f32)
        nc.vector.tensor_tensor(out=ay0[:], in0=ly1[:], in1=msk[:], op=Alu.mult)
        ay1 = sb.tile([128, PXT, 9], f32)
        nc.vector.tensor_tensor(out=ay1[:], in0=ly[:], in1=msk[:], op=Alu.mult)
        c00 = sb.tile([128, PXT, 9], f32)
        c01 = sb.tile([128, PXT, 9], f32)
        c10 = sb.tile([128, PXT, 9], f32)
        c11 = sb.tile([128, PXT, 9], f32)
        nc.vector.tensor_tensor(out=c00[:], in0=ay0[:], in1=lx1[:], op=Alu.mult)
        nc.vector.tensor_tensor(out=c01[:], in0=ay0[:], in1=lx[:], op=Alu.mult)
        nc.vector.tensor_tensor(out=c10[:], in0=ay1[:], in1=lx1[:], op=Alu.mult)
        nc.vector.tensor_tensor(out=c11[:], in0=ay1[:], in1=lx[:], op=Alu.mult)
        # clamping maps x0<=-2 (y0<=-2) pairs onto (border, image 0): the
        # second pair element then reads wrong data -> kill +1-corner coefs
        mxv = sb.tile([128, PXT, 9], f32)
        nc.vector.tensor_scalar(out=mxv[:], in0=x0f[:], scalar1=0.0,
                                scalar2=None, op0=Alu.is_ge)
        myv = sb.tile([128, PXT, 9], f32)
        nc.vector.tensor_scalar(out=myv[:], in0=y0f[:], scalar1=0.0,
                                scalar2=None, op0=Alu.is_ge)
        nc.vector.tensor_tensor(out=c01[:], in0=c01[:], in1=mxv[:], op=Alu.mult)
        nc.vector.tensor_tensor(out=c10[:], in0=c10[:], in1=myv[:], op=Alu.mult)
        nc.vector.tensor_tensor(out=c11[:], in0=c11[:], in1=mxv[:], op=Alu.mult)
        nc.vector.tensor_tensor(out=c11[:], in0=c11[:], in1=myv[:], op=Alu.mult)

        # ---------------- phase 3: wrapped int16 gather indices ------------
        # idxf [128, 9k, 2r, 16t] fp32 ; f = (k*2+r)*16 + t
        idxf = sb.tile([128, 9, 2, PXT], f32)
        top_v = swap_free(idxf[:, :, 0, :])   # iterate (t, k)
        bot_v = swap_free(idxf[:, :, 1, :])
        tmpi = sb.tile([128, PXT, 9], f32)
        nc.vector.tensor_scalar_mul(out=tmpi[:], in0=y0c[:], scalar1=66.0)
        nc.vector.tensor_tensor(out=top_v, in0=tmpi[:], in1=x0c[:], op=Alu.add)
        nc.vector.tensor_scalar_add(out=bot_v, in0=top_v, scalar1=66.0)

        # double transpose: [128p, 288f] -> [f, 128p] -> wrapped [16a, ...]
        t1sb = sb.tile([96, 3, 128], f32)
        idxf_flat = idxf[:].rearrange("p a b c -> p (a b c)")
        for chk in range(3):
            ps_t1 = ps_small.tile([96, 128], f32, tag="ps")
            nc.tensor.transpose(ps_t1[:, :],
                                idxf_flat[:, chk * 96:(chk + 1) * 96],
                                ident[:, :])
            nc.scalar.copy(out=t1sb[:, chk, :], in_=ps_t1[:, :])

        # indices must be replicated across all 8 partition groups of 16
        # (each Q7 core reads its own group) -> replication matmul with rep16
        wbuf = sb.tile([128, 18, PXT, 8], i16)
        nc.vector.memset(wbuf[:], 0)
        for chk in range(3):
            for u in range(8):
                ps_t2 = ps_small.tile([16, 96], f32, tag="ps")
                nc.tensor.transpose(ps_t2[:, :],
                                    t1sb[:, chk, u * 16:(u + 1) * 16],
                                    ident[0:96, 0:96])
                t2sb = sb.tile([16, 96], f32, tag="t2sb")
                nc.scalar.copy(out=t2sb[:], in_=ps_t2[:, :])
                ps_rep = ps_small.tile([128, 96], f32, tag="ps")
                nc.tensor.matmul(ps_rep[:, :], rep16_sb[:, :], t2sb[:, :],
                                 start=True, stop=True)
                nc.vector.tensor_copy(
                    out=wbuf[:, chk * 6:(chk + 1) * 6, :, u],
                    in_=ps_rep[:, :].rearrange("p (a t) -> p a t", t=PXT))

        # ---------------- phase 4: main loop -------------------------------
        y_sb = sb.tile([128, 2, 2, 1024], f32)      # [o', oh, half, px]
        s1b = sb.tile([128, 2, 2], f32)
        s2b = sb.tile([128, 2, 2], f32)

        y_ps = [ps_y.tile([128, 1024], f32, tag=f"y_ps{oh}", name=f"y_ps{oh}")
                for oh in range(2)]

        for half in range(2):
            for k in range(KK):
                g_top = gat.tile([128, 8, 512], f32, tag="g_top")
                g_bot = gat.tile([128, 8, 512], f32, tag="g_bot")
                nc.gpsimd.dma_gather(
                    out_ap=g_top[:], in_ap=xt_win,
                    idxs_ap=wbuf[:, k * 2, half * 8:(half + 1) * 8, :],
                    num_idxs=1024, num_idxs_reg=1024,
                    elem_size=512, elem_step=256, queue_num=0)
                nc.gpsimd.dma_gather(
                    out_ap=g_bot[:], in_ap=xt_win,
                    idxs_ap=wbuf[:, k * 2 + 1, half * 8:(half + 1) * 8, :],
                    num_idxs=1024, num_idxs_reg=1024,
                    elem_size=512, elem_step=256, queue_num=0)
                for t8 in range(8):
                    t = half * 8 + t8
                    val = vals.tile([128, 256], f32, tag="val")
                    nc.vector.tensor_scalar_mul(
                        out=val[:], in0=g_top[:, t8, 0:256],
                        scalar1=c00[:, t, k:k + 1])
                    nc.vector.scalar_tensor_tensor(
                        out=val[:], in0=g_top[:, t8, 256:512],
                        scalar=c01[:, t, k:k + 1], in1=val[:],
                        op0=Alu.mult, op1=Alu.add)
                    nc.vector.scalar_tensor_tensor(
                        out=val[:], in0=g_bot[:, t8, 0:256],
                        scalar=c10[:, t, k:k + 1], in1=val[:],
                        op0=Alu.mult, op1=Alu.add)
                    nc.vector.scalar_tensor_tensor(
                        out=val[:], in0=g_bot[:, t8, 256:512],
                        scalar=c11[:, t, k:k + 1], in1=val[:],
                        op0=Alu.mult, op1=Alu.add)
                    ps_v = ps_vt.tile([128, 256], f32, tag="ps_v")
                    nc.tensor.transpose(ps_v[:, 0:128], val[:, 0:128],
                                        ident[:, :])
                    nc.tensor.transpose(ps_v[:, 128:256], val[:, 128:256],
                                        ident[:, :])
                    valT = vals.tile([128, 256], f32, tag="valT")
                    nc.scalar.copy(out=valT[:], in_=ps_v[:])
                    for oh in range(2):
                        for cc in range(2):
                            # start/stop are per 2KB PSUM bank (= 4 t8 slices):
                            # start clears has_written for the whole bank, so
                            # only the first matmul touching the bank sets it.
                            nc.tensor.matmul(
                                y_ps[oh][:, t8 * 128:(t8 + 1) * 128],
                                w2_sb[:, k, cc, oh, :],
                                valT[:, cc * 128:(cc + 1) * 128],
                                start=(k == 0 and cc == 0 and t8 % 4 == 0),
                                stop=(k == KK - 1 and cc == 1 and t8 % 4 == 3))
            for oh in range(2):
                sq_scratch = sb.tile([128, 1024], f32, tag="sq")
                nc.scalar.activation(out=y_sb[:, oh, half, :], in_=y_ps[oh][:],
                                     func=Act.Copy,
                                     accum_out=s1b[:, oh, half:half + 1])
                nc.scalar.activation(out=sq_scratch[:], in_=y_ps[oh][:],
                                     func=Act.Square,
                                     accum_out=s2b[:, oh, half:half + 1])

        # ---------------- phase 5: GroupNorm -------------------------------
        s1 = sb.tile([128, 2], f32)
        nc.vector.tensor_tensor(out=s1[:], in0=s1b[:, :, 0], in1=s1b[:, :, 1],
                                op=Alu.add)
        s2 = sb.tile([128, 2], f32)
        nc.vector.tensor_tensor(out=s2[:], in0=s2b[:, :, 0], in1=s2b[:, :, 1],
                                op=Alu.add)
        # fold conv bias b: S1' = S1 + NPX*b ; S2' = S2 + 2 b S1 + NPX b^2
        stk = sb.tile([128, 4], f32)
        q1 = sb.tile([128, 2], f32)
        nc.vector.tensor_tensor(out=q1[:], in0=bvec_sb[:], in1=s1[:],
                                op=Alu.mult)
        nc.vector.scalar_tensor_tensor(out=stk[:, 2:4], in0=q1[:], scalar=2.0,
                                       in1=s2[:], op0=Alu.mult, op1=Alu.add)
        q2 = sb.tile([128, 2], f32)
        nc.vector.tensor_tensor(out=q2[:], in0=bvec_sb[:], in1=bvec_sb[:],
                                op=Alu.mult)
        nc.vector.scalar_tensor_tensor(out=stk[:, 2:4], in0=q2[:],
                                       scalar=float(NPX), in1=stk[:, 2:4],
                                       op0=Alu.mult, op1=Alu.add)
        nc.vector.scalar_tensor_tensor(out=stk[:, 0:2], in0=bvec_sb[:],
                                       scalar=float(NPX), in1=s1[:],
                                       op0=Alu.mult, op1=Alu.add)
        ps_s = ps_small.tile([1, 4], f32, tag="ps")
        nc.tensor.matmul(ps_s[:, :], ones_col[:, :], stk[:, :],
                         start=True, stop=True)
        tot4 = sb.tile([1, 4], f32)
        nc.vector.tensor_copy(out=tot4[:], in_=ps_s[:, :])
        ccs = sb.tile([1, 8], f32)
        nc.vector.memset(ccs[:], 0.0)
        nc.vector.tensor_tensor(out=ccs[:, 0:1], in0=tot4[:, 0:1],
                                in1=tot4[:, 1:2], op=Alu.add)
        nc.vector.tensor_tensor(out=ccs[:, 1:2], in0=tot4[:, 2:3],
                                in1=tot4[:, 3:4], op=Alu.add)

        tot = sb.tile([1, 8], f32)
        if use_collective:
            nc.sync.dma_start(out=cc_in[:], in_=ccs[:])
            nc.gpsimd.collective_compute(
                "AllReduce", Alu.add,
                replica_groups=[[0, 1], [2, 3], [4, 5], [6, 7]],
                ins=[cc_in[:].opt()], outs=[cc_out[:].opt()])
            nc.sync.dma_start(out=tot[:], in_=cc_out[:])
        else:
            nc.vector.tensor_scalar_mul(out=tot[:], in0=ccs[:], scalar1=2.0)

        invN = 1.0 / float(C * H * W)
        mu = sb.tile([1, 1], f32)
        nc.vector.tensor_scalar_mul(out=mu[:], in0=tot[:, 0:1], scalar1=invN)
        mu2 = sb.tile([1, 1], f32)
        nc.vector.tensor_tensor(out=mu2[:], in0=mu[:], in1=mu[:], op=Alu.mult)
        var = sb.tile([1, 1], f32)
        nc.vector.scalar_tensor_tensor(out=var[:], in0=tot[:, 1:2],
                                       scalar=invN, in1=mu2[:],
                                       op0=Alu.mult, op1=Alu.subtract)
        std = sb.tile([1, 1], f32)
        nc.scalar.activation(out=std[:], in_=var[:], func=Act.Sqrt,
                             bias=eps_t[:, 0:1])
        rs = sb.tile([1, 1], f32)
        nc.vector.reciprocal(out=rs[:], in_=std[:])
        mr = sb.tile([1, 2], f32)
        nc.vector.tensor_copy(out=mr[:, 0:1], in_=mu[:])
        nc.vector.tensor_copy(out=mr[:, 1:2], in_=rs[:])
        ps_b = ps_small.tile([128, 2], f32, tag="ps")
        nc.tensor.matmul(ps_b[:, :], ones_row[:, :], mr[:, :],
                         start=True, stop=True)
        mr128 = sb.tile([128, 2], f32)
        nc.vector.tensor_copy(out=mr128[:], in_=ps_b[:, :])
        svec = sb.tile([128, 2], f32)
        nc.vector.tensor_scalar_mul(out=svec[:], in0=gam_sb[:],
                                    scalar1=mr128[:, 1:2])
        tdiff = sb.tile([128, 2], f32)
        nc.vector.tensor_scalar_sub(out=tdiff[:], in0=bvec_sb[:],
                                    scalar1=mr128[:, 0:1])
        b2 = sb.tile([128, 2], f32)
        nc.vector.tensor_tensor(out=b2[:], in0=tdiff[:], in1=svec[:],
                                op=Alu.mult)
        nc.vector.tensor_tensor(out=b2[:], in0=b2[:], in1=bet_sb[:],
                                op=Alu.add)

        for oh in range(2):
            for half in range(2):
                nc.scalar.activation(out=y_sb[:, oh, half, :],
                                     in_=y_sb[:, oh, half, :],
                                     func=Act.Relu,
                                     scale=svec[:, oh:oh + 1],
                                     bias=b2[:, oh:oh + 1])
            nc.sync.dma_start(out=yout[oh * 128:(oh + 1) * 128, :],
                              in_=y_sb[:, oh, :, :])

    nc.compile()
    return nc


# ----------------------------------------------------------------- entry
def kernel(x, w_off, b_off, w, b, gamma, beta):
    from concourse.bass_utils import run_bass_kernel_spmd

    in_maps = prep_per_core(np.asarray(x, np.float32),
                            np.asarray(w_off, np.float32),
                            np.asarray(b_off, np.float32),
                            np.asarray(w, np.float32),
                            np.asarray(b, np.float32),
                            np.asarray(gamma, np.float32),
                            np.asarray(beta, np.float32))
    if "nc" not in _cache:
        _cache["nc"] = build_module(use_collective=True)
    res = run_bass_kernel_spmd(_cache["nc"], in_maps,
                               core_ids=list(range(NCORES)))
    out = np.zeros((B, CO, H, W), np.float32)
    for core in range(NCORES):
        bi, hh = core // 2, core % 2
        out[bi, :, hh * 32:(hh + 1) * 32, :] = (
            res.results[core]["yout"].reshape(CO, 32, 64))
    return out



# revision 10
# speedup vs baseline: 1.9705x; 1.9705x over previous
"""Trainium2 Bass kernel for nn_DCNConvModule (modulated deformable conv
+ GroupNorm(1) + ReLU).

Sharding: 8 cores; core (2b + h) computes sample b, output rows [32h, 32h+32).
GroupNorm statistics are per-sample -> tiny AllReduce of (sum, sumsq) within
core pairs [[0,1],[2,3],[4,5],[6,7]].

Per-core algorithm (pixel-major "px" = 2048 output pixels on 16 tiles of 128),
fp16 hot path:
  1. offset conv as pixel-major PE matmuls -> offT [px, 27] directly
     (lhsT = fp16 x-slab window, rhs = fp16 conv_offset weights).
  2. coef/index math on DVE in fp32; a zero-padded "quad" gather table in
     DRAM (one row = the 4 bilinear corner cells, 1024 fp16) makes corner
     validity masking implicit.
  3. per tap: one dma_gather fetches all 4 corners of 1024 pixels; the
     bilinear+mask combine is fused into PE transposes as diagonal-matrix
     matmuls (diag(coef) built with one 4x-mode tensor_scalar each), with
     the 4 corners accumulated in PSUM; PSUM->SBUF copies rotate across
     Act/DVE/Pool; 4 fp16 matmuls per (tile, tap) accumulate y in PSUM.
  4. GN: per-partition sums via ACT accumulators, ones-matmul partition
     reduce, pair AllReduce, normalize+ReLU as one ACT op per chunk.
"""
import contextlib
import numpy as np

K = 3
KK = 9
C = 256
CO = 256
H = 64
W = 64
B = 4
GW = 66                  # padded grid width
TQ = 4416                # quad-table rows (>= 65*66+65+1; TQ+67 <= 68*66)
PXT = 16                 # 128-pixel tiles per core
NPX = PXT * 128          # 2048 pixels per core
GN_EPS = 1e-5
NCORES = 8

_cache = {}


# ----------------------------------------------------------------- host prep
def prep_per_core(x, w_off, b_off, w, b, gamma, beta):
    """Build the 8 per-core input maps (all numpy, layout-only work)."""
    ki = np.arange(KK) // K
    kj = np.arange(KK) % K

    # conv_offset lhsT  [128, 2, 9, 27]: [c', cc, tap, o]
    woff_r = np.ascontiguousarray(
        w_off.reshape(27, 2, 128, K, K).transpose(2, 1, 3, 4, 0)
        .reshape(128, 2, 9, 27)).astype(np.float16)
    # main DCN lhsT  [128, 9, 2, 2, 128]: [c', k, cc, oh, o']
    w2 = np.ascontiguousarray(
        w.reshape(2, 128, 2, 128, KK).transpose(3, 4, 2, 0, 1)
    ).astype(np.float16)
    bvec = np.ascontiguousarray(b.reshape(2, 128).T).astype(np.float32)
    gam2 = np.ascontiguousarray(gamma.reshape(2, 128).T).astype(np.float32)
    bet2 = np.ascontiguousarray(beta.reshape(2, 128).T).astype(np.float32)

    p = np.arange(128)
    # dy/dx conv biases folded into the base sampling coordinates
    basex = (p[:, None] % 64 + kj[None, :]
             + b_off[2 * np.arange(KK) + 1][None, :]).astype(np.float32)
    # mask conv bias, replicated across partitions
    bmrep = np.broadcast_to(b_off[18:27][None, :], (128, 9)).astype(np.float32)
    bmrep = np.ascontiguousarray(bmrep)

    x_pad = np.zeros((B, C, H + 2, W + 2), np.float32)
    x_pad[:, :, 1:H+1, 1:W+1] = x

    in_maps = []
    for core in range(NCORES):
        bi, hh = core // 2, core % 2
        h0 = hh * 32
        # quad table: row i = cells [i, i+1, i+66, i+67] of the flat padded
        # grid (row-major 66 wide, rows 66-67 zero) -> all 4 bilinear
        # corners of (y0, x0) with implicit zero at every wrap.
        grid = np.zeros((68, GW, C), np.float32)
        grid[:66, :, :] = x_pad[bi].transpose(1, 2, 0)
        flat = grid.reshape(68 * GW, C)
        quad = np.concatenate(
            [flat[0:TQ], flat[1:TQ+1], flat[66:TQ+66], flat[67:TQ+67]],
            axis=1).astype(np.float16)
        slab = x_pad[bi][:, h0:h0+34, :]                    # [256, 34, 66]
        # three column-shifted 64-wide copies: any (row-pair, tap) window of
        # 128 pixels is then contiguous (stationary matmul operands must
        # have a single free dim)
        sl3 = np.stack([slab[:, :, bb:bb+64] for bb in range(3)], axis=1)
        xc = np.ascontiguousarray(
            sl3.reshape(2, 128, 3, 34 * 64).transpose(1, 0, 2, 3)
        ).astype(np.float16)
        t = np.arange(PXT)
        basey = np.ascontiguousarray(
            (h0 + (t[None, :, None] * 128 + p[:, None, None]) // 64
             + ki[None, None, :])
            + b_off[2 * np.arange(KK)][None, None, :]).astype(np.float32)
        rep = np.zeros((16, 128), np.float32)
        rep[np.arange(128) % 16, np.arange(128)] = 1.0
        in_maps.append(dict(
            xt=np.ascontiguousarray(quad), xc=xc,
            woff=woff_r, w2=w2,
            bvec=bvec, gam2=gam2, bet2=bet2, bmrep=bmrep,
            basey=basey, basex=basex, rep16=rep,
        ))
    return in_maps


# --------------------------------------------------------------- bass kernel
def build_module(use_collective=True):
    import concourse.bacc as bacc
    import concourse.bass as bass
    import concourse.tile as tile
    from concourse import mybir
    from concourse.masks import make_identity

    f32 = mybir.dt.float32
    f16 = mybir.dt.float16
    i16 = mybir.dt.int16
    Alu = mybir.AluOpType
    Act = mybir.ActivationFunctionType

    nc = bacc.Bacc("TRN2", target_bir_lowering=False, debug=False,
                   num_devices=NCORES)

    xt = nc.dram_tensor("xt", [TQ, 1024], f16, kind="ExternalInput")
    xc = nc.dram_tensor("xc", [128, 2, 3, 34 * 64], f16, kind="ExternalInput")
    woff = nc.dram_tensor("woff", [128, 2, 9, 27], f16, kind="ExternalInput")
    w2 = nc.dram_tensor("w2", [128, 9, 2, 2, 128], f16, kind="ExternalInput")
    bvec = nc.dram_tensor("bvec", [128, 2], f32, kind="ExternalInput")
    gam2 = nc.dram_tensor("gam2", [128, 2], f32, kind="ExternalInput")
    bet2 = nc.dram_tensor("bet2", [128, 2], f32, kind="ExternalInput")
    bmrep = nc.dram_tensor("bmrep", [128, 9], f32, kind="ExternalInput")
    basey = nc.dram_tensor("basey", [128, PXT, 9], f32, kind="ExternalInput")
    basex = nc.dram_tensor("basex", [128, 9], f32, kind="ExternalInput")
    rep16 = nc.dram_tensor("rep16", [16, 128], f32, kind="ExternalInput")
    yout = nc.dram_tensor("yout", [CO, NPX], f32, kind="ExternalOutput")

    cc_in = nc.dram_tensor("cc_in", [1, 8], f32)
    cc_out = nc.dram_tensor("cc_out", [1, 8], f32)

    xt_win = bass.AP(tensor=xt, offset=0, ap=[[1024, TQ], [1, 1024]])

    def swap_free(ap2):
        """Swap the two free dims of a [P, A, B] AP (iteration order only)."""
        return bass.AP(tensor=ap2.tensor, offset=ap2.offset,
                       ap=[ap2.ap[0], ap2.ap[2], ap2.ap[1]])

    def bcast_free(ap1, n):
        """View a [P, F] AP as [P, n, F] with stride-0 broadcast."""
        return bass.AP(tensor=ap1.tensor, offset=ap1.offset,
                       ap=[ap1.ap[0], [0, n], ap1.ap[1]])

    with tile.TileContext(nc) as tc, contextlib.ExitStack() as ctx:
        consts = ctx.enter_context(tc.tile_pool(name="consts", bufs=1))
        sb = ctx.enter_context(tc.tile_pool(name="sb", bufs=1))
        ps_small = ctx.enter_context(
            tc.tile_pool(name="ps_small", bufs=1, space="PSUM"))
        gat = ctx.enter_context(tc.tile_pool(name="gat", bufs=2))
        diags = ctx.enter_context(tc.tile_pool(name="diags", bufs=2))
        vals = ctx.enter_context(tc.tile_pool(name="vals", bufs=3))
        ps_vt = ctx.enter_context(
            tc.tile_pool(name="ps_vt", bufs=3, space="PSUM"))
        ps_y = ctx.enter_context(
            tc.tile_pool(name="ps_y", bufs=1, space="PSUM"))

        ident32 = consts.tile([128, 128], f32)
        make_identity(nc, ident32[:])
        ident16 = consts.tile([128, 128], f16)
        make_identity(nc, ident16[:])
        ones_row = consts.tile([1, 128], f32)
        nc.vector.memset(ones_row[:], 1.0)
        ones_col = consts.tile([128, 1], f32)
        nc.vector.memset(ones_col[:], 1.0)
        eps_t = consts.tile([1, 1], f32)
        nc.vector.memset(eps_t[:], GN_EPS)

        xc_sb = consts.tile([128, 2, 3, 34 * 64], f16)
        nc.sync.dma_start(out=xc_sb[:], in_=xc[:])
        woff_sb = consts.tile([128, 2, 9, 27], f16)
        nc.sync.dma_start(out=woff_sb[:], in_=woff[:])
        w2_sb = consts.tile([128, 9, 2, 2, 128], f16)
        nc.sync.dma_start(out=w2_sb[:], in_=w2[:])
        bvec_sb = consts.tile([128, 2], f32)
        nc.sync.dma_start(out=bvec_sb[:], in_=bvec[:])
        gam_sb = consts.tile([128, 2], f32)
        nc.sync.dma_start(out=gam_sb[:], in_=gam2[:])
        bet_sb = consts.tile([128, 2], f32)
        nc.sync.dma_start(out=bet_sb[:], in_=bet2[:])
        bmrep_sb = consts.tile([128, 9], f32)
        nc.sync.dma_start(out=bmrep_sb[:], in_=bmrep[:])
        basey_sb = consts.tile([128, PXT, 9], f32)
        nc.sync.dma_start(out=basey_sb[:], in_=basey[:])
        basex_sb = consts.tile([128, 9], f32)
        nc.sync.dma_start(out=basex_sb[:], in_=basex[:])
        rep16_sb = consts.tile([16, 128], f32)
        nc.sync.dma_start(out=rep16_sb[:], in_=rep16[:])

        # ------------- phase 1: offset conv, pixel-major  offT[px, 27] ------
        offT = sb.tile([128, PXT, 27], f32)
        for t in range(PXT):
            ps_off = ps_small.tile([128, 27], f32, tag="ps")
            n = 0
            for cc in range(2):
                for a in range(3):
                    for bb in range(3):
                        r0 = (2 * t + a) * 64
                        nc.tensor.matmul(
                            ps_off[:, :],
                            xc_sb[:, cc, bb, r0: r0 + 128],
                            woff_sb[:, cc, a * 3 + bb, :],
                            start=(n == 0), stop=(n == 17))
                        n += 1
            nc.scalar.copy(out=offT[:, t, :], in_=ps_off[:, :])

        # ---------------- phase 2: coef + index math (batched) -------------
        dy = offT[:, :, 0:18:2]     # [128, 16, 9] strided views
        dx = offT[:, :, 1:18:2]
        moff = offT[:, :, 18:27]

        mss = sb.tile([128, PXT, 9], f32)
        nc.vector.tensor_tensor(out=mss[:], in0=moff,
                                in1=bcast_free(bmrep_sb[:], PXT), op=Alu.add)
        msk = sb.tile([128, PXT, 9], f32)
        nc.scalar.activation(out=msk[:], in_=mss[:], func=Act.Sigmoid)

        pyg = sb.tile([128, PXT, 9], f32)
        nc.vector.tensor_tensor(out=pyg[:], in0=dy, in1=basey_sb[:], op=Alu.add)
        pxg = sb.tile([128, PXT, 9], f32)
        nc.vector.tensor_tensor(out=pxg[:], in0=dx,
                                in1=bcast_free(basex_sb[:], PXT), op=Alu.add)

        def floor_(src, dst_f, dst_frac, dst_clamp):
            ii = sb.tile([128, PXT, 9], mybir.dt.int32, tag="flo_i")
            nc.vector.tensor_copy(out=ii[:], in_=src[:])
            ff = sb.tile([128, PXT, 9], f32, tag="flo_f")
            nc.vector.tensor_copy(out=ff[:], in_=ii[:])
            gt = sb.tile([128, PXT, 9], f32, tag="flo_g")
            nc.vector.tensor_tensor(out=gt[:], in0=ff[:], in1=src[:],
                                    op=Alu.is_gt)
            nc.vector.tensor_tensor(out=dst_f[:], in0=ff[:], in1=gt[:],
                                    op=Alu.subtract)
            nc.vector.tensor_tensor(out=dst_frac[:], in0=src[:], in1=dst_f[:],
                                    op=Alu.subtract)
            nc.vector.tensor_scalar(out=dst_clamp[:], in0=dst_f[:],
                                    scalar1=0.0, scalar2=65.0,
                                    op0=Alu.max, op1=Alu.min)

        y0f = sb.tile([128, PXT, 9], f32)
        ly = sb.tile([128, PXT, 9], f32)
        y0c = sb.tile([128, PXT, 9], f32)
        floor_(pyg, y0f, ly, y0c)
        x0f = sb.tile([128, PXT, 9], f32)
        lx = sb.tile([128, PXT, 9], f32)
        x0c = sb.tile([128, PXT, 9], f32)
        floor_(pxg, x0f, lx, x0c)

        ly1 = sb.tile([128, PXT, 9], f32)
        nc.vector.tensor_scalar(out=ly1[:], in0=ly[:], scalar1=-1.0,
                                scalar2=1.0, op0=Alu.mult, op1=Alu.add)
        lx1 = sb.tile([128, PXT, 9], f32)
        nc.vector.tensor_scalar(out=lx1[:], in0=lx[:], scalar1=-1.0,
                                scalar2=1.0, op0=Alu.mult, op1=Alu.add)
        ay0 = sb.tile([128, PXT, 9], f32)
        nc.vector.tensor_tensor(out=ay0[:], in0=ly1[:], in1=msk[:], op=Alu.mult)
        ay1 = sb.tile([128, PXT, 9], f32)
        nc.vector.tensor_tensor(out=ay1[:], in0=ly[:], in1=msk[:], op=Alu.mult)
        c00 = sb.tile([128, PXT, 9], f32)
        c01 = sb.tile([128, PXT, 9], f32)
        c10 = sb.tile([128, PXT, 9], f32)
        c11 = sb.tile([128, PXT, 9], f32)
        nc.vector.tensor_tensor(out=c00[:], in0=ay0[:], in1=lx1[:], op=Alu.mult)
        nc.vector.tensor_tensor(out=c01[:], in0=ay0[:], in1=lx[:], op=Alu.mult)
        nc.vector.tensor_tensor(out=c10[:], in0=ay1[:], in1=lx1[:], op=Alu.mult)
        nc.vector.tensor_tensor(out=c11[:], in0=ay1[:], in1=lx[:], op=Alu.mult)
        # clamping maps x0<=-2 (y0<=-2) pairs onto (border, image 0): the
        # second pair element then reads wrong data -> kill +1-corner coefs
        mxv = sb.tile([128, PXT, 9], f32)
        nc.vector.tensor_scalar(out=mxv[:], in0=x0f[:], scalar1=0.0,
                                scalar2=None, op0=Alu.is_ge)
        myv = sb.tile([128, PXT, 9], f32)
        nc.vector.tensor_scalar(out=myv[:], in0=y0f[:], scalar1=0.0,
                                scalar2=None, op0=Alu.is_ge)
        nc.vector.tensor_tensor(out=c01[:], in0=c01[:], in1=mxv[:], op=Alu.mult)
        nc.vector.tensor_tensor(out=c10[:], in0=c10[:], in1=myv[:], op=Alu.mult)
        nc.vector.tensor_tensor(out=c11[:], in0=c11[:], in1=mxv[:], op=Alu.mult)
        nc.vector.tensor_tensor(out=c11[:], in0=c11[:], in1=myv[:], op=Alu.mult)

        # ---------------- phase 3: wrapped int16 gather indices ------------
        # idxf [128, 9k, 16t] fp32 ; f = k*16 + t
        idxf = sb.tile([128, 9, PXT], f32)
        idx_v = swap_free(idxf[:])            # iterate (t, k)
        tmpi = sb.tile([128, PXT, 9], f32)
        nc.vector.tensor_scalar_mul(out=tmpi[:], in0=y0c[:], scalar1=66.0)
        nc.vector.tensor_tensor(out=idx_v, in0=tmpi[:], in1=x0c[:], op=Alu.add)

        # double transpose: [128p, 144f] -> [f, 128p] -> wrapped [16a, ...]
        idxf_flat = idxf[:].rearrange("p a b -> p (a b)")
        chunks = [(0, 96), (96, 48)]
        t1sb = sb.tile([96, 2, 128], f32)
        for chk, (c0, cw) in enumerate(chunks):
            ps_t1 = ps_small.tile([96, 128], f32, tag="ps")
            nc.tensor.transpose(ps_t1[0:cw, :],
                                idxf_flat[:, c0:c0 + cw],
                                ident32[:, :])
            nc.scalar.copy(out=t1sb[0:cw, chk, :], in_=ps_t1[0:cw, :])

        # indices must be replicated across all 8 partition groups of 16
        # (each Q7 core reads its own group) -> replication matmul with rep16
        wbuf = sb.tile([128, 9, PXT, 8], i16)
        for chk, (c0, cw) in enumerate(chunks):
            nfr = cw // 16
            for u in range(8):
                ps_t2 = ps_small.tile([16, 96], f32, tag="ps")
                nc.tensor.transpose(ps_t2[:, 0:cw],
                                    t1sb[0:cw, chk, u * 16:(u + 1) * 16],
                                    ident32[0:cw, 0:cw])
                t2sb = sb.tile([16, 96], f32, tag="t2sb")
                nc.scalar.copy(out=t2sb[:, 0:cw], in_=ps_t2[:, 0:cw])
                ps_rep = ps_small.tile([128, 96], f32, tag="ps")
                nc.tensor.matmul(ps_rep[:, 0:cw], rep16_sb[:, :],
                                 t2sb[:, 0:cw], start=True, stop=True)
                nc.vector.tensor_copy(
                    out=wbuf[:, c0 // 16:c0 // 16 + nfr, :, u],
                    in_=ps_rep[:, 0:cw].rearrange("p (a t) -> p a t", t=PXT))

        # ---------------- phase 4: main loop -------------------------------
        y_sb = sb.tile([128, 2, 2, 1024], f32)      # [o', oh, half, px]
        s1b = sb.tile([128, 2, 2], f32)
        s2b = sb.tile([128, 2, 2], f32)

        y_ps = [ps_y.tile([128, 1024], f32, tag=f"y_ps{oh}", name=f"y_ps{oh}")
                for oh in range(2)]

        # GPSIMD cannot access PSUM -> rotate copies across Act and DVE
        cp_engines = [nc.scalar.copy, nc.vector.tensor_copy]
        cp_i = 0

        for half in range(2):
            for k in range(KK):
                g = gat.tile([128, 8, 1024], f16, tag="g")
                nc.gpsimd.dma_gather(
                    out_ap=g[:], in_ap=xt_win,
                    idxs_ap=wbuf[:, k, half * 8:(half + 1) * 8, :],
                    num_idxs=1024, num_idxs_reg=1024,
                    elem_size=1024, queue_num=0)
                for t8p in range(4):                 # pairs of px tiles
                    ps_v = ps_vt.tile([128, 512], f32, tag="ps_v")
                    dg = [[diags.tile([128, 128], f16, tag=f"d{j}{cn}",
                                      name=f"d{j}{cn}")
                           for cn in range(4)] for j in range(2)]
                    for j in range(2):
                        t8 = t8p * 2 + j
                        t = half * 8 + t8
                        for cn, cf in enumerate((c00, c01, c10, c11)):
                            nc.vector.tensor_scalar_mul(
                                out=dg[j][cn][:], in0=ident16[:],
                                scalar1=cf[:, t, k:k + 1])
                    for j in range(2):
                        t8 = t8p * 2 + j
                        for hh in range(2):
                            for cn in range(4):
                                nc.tensor.matmul(
                                    ps_v[:, j * 256 + hh * 128:
                                         j * 256 + (hh + 1) * 128],
                                    g[:, t8, cn * 256 + hh * 128:
                                      cn * 256 + (hh + 1) * 128],
                                    dg[j][cn][:],
                                    start=(j == 0 and hh == 0 and cn == 0),
                                    stop=(j == 1 and hh == 1 and cn == 3))
                    valT = vals.tile([128, 512], f16, tag="valT")
                    cp_engines[cp_i % 2](out=valT[:], in_=ps_v[:])
                    cp_i += 1
                    for j in range(2):
                        t8 = t8p * 2 + j
                        for oh in range(2):
                            for cc in range(2):
                                # start/stop are per 2KB PSUM bank (= 4 t8
                                # slices): start clears has_written for the
                                # whole bank, so only the first matmul
                                # touching the bank sets it.
                                nc.tensor.matmul(
                                    y_ps[oh][:, t8 * 128:(t8 + 1) * 128],
                                    w2_sb[:, k, cc, oh, :],
                                    valT[:, j * 256 + cc * 128:
                                         j * 256 + (cc + 1) * 128],
                                    start=(k == 0 and cc == 0 and t8 % 4 == 0),
                                    stop=(k == KK - 1 and cc == 1
                                          and t8 % 4 == 3))
            for oh in range(2):
                sq_scratch = sb.tile([128, 1024], f32, tag="sq")
                nc.scalar.activation(out=y_sb[:, oh, half, :], in_=y_ps[oh][:],
                                     func=Act.Copy,
                                     accum_out=s1b[:, oh, half:half + 1])
                nc.scalar.activation(out=sq_scratch[:], in_=y_ps[oh][:],
                                     func=Act.Square,
                                     accum_out=s2b[:, oh, half:half + 1])

        # ---------------- phase 5: GroupNorm -------------------------------
        s1 = sb.tile([128, 2], f32)
        nc.vector.tensor_tensor(out=s1[:], in0=s1b[:, :, 0], in1=s1b[:, :, 1],
                                op=Alu.add)
        s2 = sb.tile([128, 2], f32)
        nc.vector.tensor_tensor(out=s2[:], in0=s2b[:, :, 0], in1=s2b[:, :, 1],
                                op=Alu.add)
        # fold conv bias b: S1' = S1 + NPX*b ; S2' = S2 + 2 b S1 + NPX b^2
        stk = sb.tile([128, 4], f32)
        q1 = sb.tile([128, 2], f32)
        nc.vector.tensor_tensor(out=q1[:], in0=bvec_sb[:], in1=s1[:],
                                op=Alu.mult)
        nc.vector.scalar_tensor_tensor(out=stk[:, 2:4], in0=q1[:], scalar=2.0,
                                       in1=s2[:], op0=Alu.mult, op1=Alu.add)
        q2 = sb.tile([128, 2], f32)
        nc.vector.tensor_tensor(out=q2[:], in0=bvec_sb[:], in1=bvec_sb[:],
                                op=Alu.mult)
        nc.vector.scalar_tensor_tensor(out=stk[:, 2:4], in0=q2[:],
                                       scalar=float(NPX), in1=stk[:, 2:4],
                                       op0=Alu.mult, op1=Alu.add)
        nc.vector.scalar_tensor_tensor(out=stk[:, 0:2], in0=bvec_sb[:],
                                       scalar=float(NPX), in1=s1[:],
                                       op0=Alu.mult, op1=Alu.add)
        ps_s = ps_small.tile([1, 4], f32, tag="ps")
        nc.tensor.matmul(ps_s[:, :], ones_col[:, :], stk[:, :],
                         start=True, stop=True)
        tot4 = sb.tile([1, 4], f32)
        nc.vector.tensor_copy(out=tot4[:], in_=ps_s[:, :])
        ccs = sb.tile([1, 8], f32)
        nc.vector.memset(ccs[:], 0.0)
        nc.vector.tensor_tensor(out=ccs[:, 0:1], in0=tot4[:, 0:1],
                                in1=tot4[:, 1:2], op=Alu.add)
        nc.vector.tensor_tensor(out=ccs[:, 1:2], in0=tot4[:, 2:3],
                                in1=tot4[:, 3:4], op=Alu.add)

        tot = sb.tile([1, 8], f32)
        if use_collective:
            nc.sync.dma_start(out=cc_in[:], in_=ccs[:])
            nc.gpsimd.collective_compute(
                "AllReduce", Alu.add,
                replica_groups=[[0, 1], [2, 3], [4, 5], [6, 7]],
                ins=[cc_in[:].opt()], outs=[cc_out[:].opt()])
            nc.sync.dma_start(out=tot[:], in_=cc_out[:])
        else:
            nc.vector.tensor_scalar_mul(out=tot[:], in0=ccs[:], scalar1=2.0)

        invN = 1.0 / float(C * H * W)
        mu = sb.tile([1, 1], f32)
        nc.vector.tensor_scalar_mul(out=mu[:], in0=tot[:, 0:1], scalar1=invN)
        mu2 = sb.tile([1, 1], f32)
        nc.vector.tensor_tensor(out=mu2[:], in0=mu[:], in1=mu[:], op=Alu.mult)
        var = sb.tile([1, 1], f32)
        nc.vector.scalar_tensor_tensor(out=var[:], in0=tot[:, 1:2],
                                       scalar=invN, in1=mu2[:],
                                       op0=Alu.mult, op1=Alu.subtract)
        std = sb.tile([1, 1], f32)
        nc.scalar.activation(out=std[:], in_=var[:], func=Act.Sqrt,
                             bias=eps_t[:, 0:1])
        rs = sb.tile([1, 1], f32)
        nc.vector.reciprocal(out=rs[:], in_=std[:])
        mr = sb.tile([1, 2], f32)
        nc.vector.tensor_copy(out=mr[:, 0:1], in_=mu[:])
        nc.vector.tensor_copy(out=mr[:, 1:2], in_=rs[:])
        ps_b = ps_small.tile([128, 2], f32, tag="ps")
        nc.tensor.matmul(ps_b[:, :], ones_row[:, :], mr[:, :],
                         start=True, stop=True)
        mr128 = sb.tile([128, 2], f32)
        nc.vector.tensor_copy(out=mr128[:], in_=ps_b[:, :])
        svec = sb.tile([128, 2], f32)
        nc.vector.tensor_scalar_mul(out=svec[:], in0=gam_sb[:],
                                    scalar1=mr128[:, 1:2])
        tdiff = sb.tile([128, 2], f32)
        nc.vector.tensor_scalar_sub(out=tdiff[:], in0=bvec_sb[:],
                                    scalar1=mr128[:, 0:1])
        b2 = sb.tile([128, 2], f32)
        nc.vector.tensor_tensor(out=b2[:], in0=tdiff[:], in1=svec[:],
                                op=Alu.mult)
        nc.vector.tensor_tensor(out=b2[:], in0=b2[:], in1=bet_sb[:],
                                op=Alu.add)

        for oh in range(2):
            for half in range(2):
                nc.scalar.activation(out=y_sb[:, oh, half, :],
                                     in_=y_sb[:, oh, half, :],
                                     func=Act.Relu,
                                     scale=svec[:, oh:oh + 1],
                                     bias=b2[:, oh:oh + 1])
            nc.sync.dma_start(out=yout[oh * 128:(oh + 1) * 128, :],
                              in_=y_sb[:, oh, :, :])

    nc.compile()
    return nc


# ----------------------------------------------------------------- entry
def kernel(x, w_off, b_off, w, b, gamma, beta):
    from concourse.bass_utils import run_bass_kernel_spmd

    in_maps = prep_per_core(np.asarray(x, np.float32),
                            np.asarray(w_off, np.float32),
                            np.asarray(b_off, np.float32),
                            np.asarray(w, np.float32),
                            np.asarray(b, np.float32),
                            np.asarray(gamma, np.float32),
                            np.asarray(beta, np.float32))
    if "nc" not in _cache:
        _cache["nc"] = build_module(use_collective=True)
    res = run_bass_kernel_spmd(_cache["nc"], in_maps,
                               core_ids=list(range(NCORES)))
    out = np.zeros((B, CO, H, W), np.float32)
    for core in range(NCORES):
        bi, hh = core // 2, core % 2
        out[bi, :, hh * 32:(hh + 1) * 32, :] = (
            res.results[core]["yout"].reshape(CO, 32, 64))
    return out


# revision 17
# speedup vs baseline: 2.1154x; 1.0735x over previous
"""Trainium2 Bass kernel for nn_DCNConvModule (modulated deformable conv
+ GroupNorm(1) + ReLU).

Sharding: 8 cores; core (2b + h) computes sample b, output rows [32h, 32h+32).
GroupNorm statistics are per-sample -> tiny AllReduce of (sum, sumsq) within
core pairs [[0,1],[2,3],[4,5],[6,7]].

Per-core algorithm (pixel-major "px" = 2048 output pixels on 16 tiles of 128),
fp16 hot path:
  1. offset conv as pixel-major PE matmuls -> offT [px, 27] directly
     (lhsT = fp16 x-slab window, rhs = fp16 conv_offset weights).
  2. coef/index math on DVE in fp32; a zero-padded "quad" gather table in
     DRAM (one row = the 4 bilinear corner cells, 1024 fp16) makes corner
     validity masking implicit.
  3. per tap: one dma_gather fetches all 4 corners of 1024 pixels; the
     bilinear+mask combine is fused into PE transposes as diagonal-matrix
     matmuls (diag(coef) built with one 4x-mode tensor_scalar each), with
     the 4 corners accumulated in PSUM; PSUM->SBUF copies rotate across
     Act/DVE/Pool; 4 fp16 matmuls per (tile, tap) accumulate y in PSUM.
  4. GN: per-partition sums via ACT accumulators, ones-matmul partition
     reduce, pair AllReduce, normalize+ReLU as one ACT op per chunk.
"""
import contextlib
import numpy as np

K = 3
KK = 9
C = 256
CO = 256
H = 64
W = 64
B = 4
GW = 66                  # padded grid width
TQ = 4416                # quad-table rows (>= 65*66+65+1; TQ+67 <= 68*66)
PXT = 16                 # 128-pixel tiles per core
NPX = PXT * 128          # 2048 pixels per core
GN_EPS = 1e-5
NCORES = 8

_cache = {}


# ----------------------------------------------------------------- host prep
def prep_per_core(x, w_off, b_off, w, b, gamma, beta):
    """Build the 8 per-core input maps (all numpy, layout-only work)."""
    ki = np.arange(KK) // K
    kj = np.arange(KK) % K

    # conv_offset lhsT  [128, 2, 9, 27]: [c', cc, tap, o]
    woff_r = np.ascontiguousarray(
        w_off.reshape(27, 2, 128, K, K).transpose(2, 1, 3, 4, 0)
        .reshape(128, 2, 9, 27)).astype(np.float16)
    # main DCN lhsT  [128, 9, 2, 2, 128]: [c', k, cc, oh, o']
    w2 = np.ascontiguousarray(
        w.reshape(2, 128, 2, 128, KK).transpose(3, 4, 2, 0, 1)
    ).astype(np.float16)
    bvec = np.ascontiguousarray(b.reshape(2, 128).T).astype(np.float32)
    gam2 = np.ascontiguousarray(gamma.reshape(2, 128).T).astype(np.float32)
    bet2 = np.ascontiguousarray(beta.reshape(2, 128).T).astype(np.float32)

    p = np.arange(128)
    # dy/dx conv biases folded into the base sampling coordinates
    basex = (p[:, None] % 64 + kj[None, :]
             + b_off[2 * np.arange(KK) + 1][None, :]).astype(np.float32)
    # mask conv bias, replicated across partitions
    bmrep = np.broadcast_to(b_off[18:27][None, :], (128, 9)).astype(np.float32)
    bmrep = np.ascontiguousarray(bmrep)

    x_pad = np.zeros((B, C, H + 2, W + 2), np.float32)
    x_pad[:, :, 1:H+1, 1:W+1] = x

    in_maps = []
    for core in range(NCORES):
        bi, hh = core // 2, core % 2
        h0 = hh * 32
        # quad table: row i = cells [i, i+1, i+66, i+67] of the flat padded
        # grid (row-major 66 wide, rows 66-67 zero) -> all 4 bilinear
        # corners of (y0, x0) with implicit zero at every wrap.
        grid = np.zeros((68, GW, C), np.float32)
        grid[:66, :, :] = x_pad[bi].transpose(1, 2, 0)
        flat = grid.reshape(68 * GW, C)
        quad = np.concatenate(
            [flat[0:TQ], flat[1:TQ+1], flat[66:TQ+66], flat[67:TQ+67]],
            axis=1).astype(np.float16)
        slab = x_pad[bi][:, h0:h0+34, :]                    # [256, 34, 66]
        # three column-shifted 64-wide copies: any (row-pair, tap) window of
        # 128 pixels is then contiguous (stationary matmul operands must
        # have a single free dim)
        sl3 = np.stack([slab[:, :, bb:bb+64] for bb in range(3)], axis=1)
        xc = np.ascontiguousarray(
            sl3.reshape(2, 128, 3, 34 * 64).transpose(1, 0, 2, 3)
        ).astype(np.float16)
        t = np.arange(PXT)
        basey = np.ascontiguousarray(
            (h0 + (t[None, :, None] * 128 + p[:, None, None]) // 64
             + ki[None, None, :])
            + b_off[2 * np.arange(KK)][None, None, :]).astype(np.float32)
        rep = np.zeros((16, 128), np.float32)
        rep[np.arange(128) % 16, np.arange(128)] = 1.0
        in_maps.append(dict(
            xt=np.ascontiguousarray(quad), xc=xc,
            woff=woff_r, w2=w2,
            bvec=bvec, gam2=gam2, bet2=bet2, bmrep=bmrep,
            basey=basey, basex=basex, rep16=rep,
        ))
    return in_maps


# --------------------------------------------------------------- bass kernel
def build_module(use_collective=True):
    import concourse.bacc as bacc
    import concourse.bass as bass
    import concourse.tile as tile
    from concourse import mybir
    from concourse.masks import make_identity

    f32 = mybir.dt.float32
    f16 = mybir.dt.float16
    i16 = mybir.dt.int16
    Alu = mybir.AluOpType
    Act = mybir.ActivationFunctionType

    nc = bacc.Bacc("TRN2", target_bir_lowering=False, debug=False,
                   num_devices=NCORES)

    xt = nc.dram_tensor("xt", [TQ, 1024], f16, kind="ExternalInput")
    xc = nc.dram_tensor("xc", [128, 2, 3, 34 * 64], f16, kind="ExternalInput")
    woff = nc.dram_tensor("woff", [128, 2, 9, 27], f16, kind="ExternalInput")
    w2 = nc.dram_tensor("w2", [128, 9, 2, 2, 128], f16, kind="ExternalInput")
    bvec = nc.dram_tensor("bvec", [128, 2], f32, kind="ExternalInput")
    gam2 = nc.dram_tensor("gam2", [128, 2], f32, kind="ExternalInput")
    bet2 = nc.dram_tensor("bet2", [128, 2], f32, kind="ExternalInput")
    bmrep = nc.dram_tensor("bmrep", [128, 9], f32, kind="ExternalInput")
    basey = nc.dram_tensor("basey", [128, PXT, 9], f32, kind="ExternalInput")
    basex = nc.dram_tensor("basex", [128, 9], f32, kind="ExternalInput")
    rep16 = nc.dram_tensor("rep16", [16, 128], f32, kind="ExternalInput")
    yout = nc.dram_tensor("yout", [CO, NPX], f32, kind="ExternalOutput")

    cc_in = nc.dram_tensor("cc_in", [1, 8], f32)
    cc_out = nc.dram_tensor("cc_out", [1, 8], f32)

    xt_win = bass.AP(tensor=xt, offset=0, ap=[[1024, TQ], [1, 1024]])

    def swap_free(ap2):
        """Swap the two free dims of a [P, A, B] AP (iteration order only)."""
        return bass.AP(tensor=ap2.tensor, offset=ap2.offset,
                       ap=[ap2.ap[0], ap2.ap[2], ap2.ap[1]])

    def bcast_free(ap1, n):
        """View a [P, F] AP as [P, n, F] with stride-0 broadcast."""
        return bass.AP(tensor=ap1.tensor, offset=ap1.offset,
                       ap=[ap1.ap[0], [0, n], ap1.ap[1]])

    with tile.TileContext(nc) as tc, contextlib.ExitStack() as ctx:
        consts = ctx.enter_context(tc.tile_pool(name="consts", bufs=1))
        sb = ctx.enter_context(tc.tile_pool(name="sb", bufs=1))
        ps_small = ctx.enter_context(
            tc.tile_pool(name="ps_small", bufs=1, space="PSUM"))
        gat = ctx.enter_context(tc.tile_pool(name="gat", bufs=3))
        diags = ctx.enter_context(tc.tile_pool(name="diags", bufs=2))
        vals = ctx.enter_context(tc.tile_pool(name="vals", bufs=3))
        ps_vt = ctx.enter_context(
            tc.tile_pool(name="ps_vt", bufs=3, space="PSUM"))
        ps_y = ctx.enter_context(
            tc.tile_pool(name="ps_y", bufs=1, space="PSUM"))

        ident32 = consts.tile([128, 128], f32)
        make_identity(nc, ident32[:])
        ident16 = consts.tile([128, 128], f16)
        make_identity(nc, ident16[:])
        ones_row = consts.tile([1, 128], f32)
        nc.vector.memset(ones_row[:], 1.0)
        ones_col = consts.tile([128, 1], f32)
        nc.vector.memset(ones_col[:], 1.0)
        eps_t = consts.tile([1, 1], f32)
        nc.vector.memset(eps_t[:], GN_EPS)

        # load order: xc+woff gate phase 1, basey/basex/rep16 gate the index
        # path to the first gather; w2 is not needed until the main matmuls
        xc_sb = consts.tile([128, 2, 3, 34 * 64], f16)
        nc.sync.dma_start(out=xc_sb[:], in_=xc[:])
        woff_sb = consts.tile([128, 2, 9, 27], f16)
        nc.sync.dma_start(out=woff_sb[:], in_=woff[:])
        basey_sb = consts.tile([128, PXT, 9], f32)
        nc.sync.dma_start(out=basey_sb[:], in_=basey[:])
        basex_sb = consts.tile([128, 9], f32)
        nc.sync.dma_start(out=basex_sb[:], in_=basex[:])
        rep16_sb = consts.tile([16, 128], f32)
        nc.sync.dma_start(out=rep16_sb[:], in_=rep16[:])
        bmrep_sb = consts.tile([128, 9], f32)
        nc.sync.dma_start(out=bmrep_sb[:], in_=bmrep[:])
        bvec_sb = consts.tile([128, 2], f32)
        nc.sync.dma_start(out=bvec_sb[:], in_=bvec[:])
        gam_sb = consts.tile([128, 2], f32)
        nc.sync.dma_start(out=gam_sb[:], in_=gam2[:])
        bet_sb = consts.tile([128, 2], f32)
        nc.sync.dma_start(out=bet_sb[:], in_=bet2[:])
        w2_sb = consts.tile([128, 9, 2, 2, 128], f16)
        nc.sync.dma_start(out=w2_sb[:], in_=w2[:])

        # ------------- phase 1: offset conv, pixel-major  offT[px, 27] ------
        offT = sb.tile([128, PXT, 27], f32)
        for t in range(PXT):
            ps_off = ps_small.tile([128, 27], f32, tag="ps")
            n = 0
            for cc in range(2):
                for a in range(3):
                    for bb in range(3):
                        r0 = (2 * t + a) * 64
                        nc.tensor.matmul(
                            ps_off[:, :],
                            xc_sb[:, cc, bb, r0: r0 + 128],
                            woff_sb[:, cc, a * 3 + bb, :],
                            start=(n == 0), stop=(n == 17))
                        n += 1
            nc.scalar.copy(out=offT[:, t, :], in_=ps_off[:, :])

        # ---------------- phase 2a: sampling coordinates -------------------
        dy = offT[:, :, 0:18:2]     # [128, 16, 9] strided views
        dx = offT[:, :, 1:18:2]
        moff = offT[:, :, 18:27]

        pyg = sb.tile([128, PXT, 9], f32)
        nc.vector.tensor_tensor(out=pyg[:], in0=dy, in1=basey_sb[:], op=Alu.add)
        pxg = sb.tile([128, PXT, 9], f32)
        nc.vector.tensor_tensor(out=pxg[:], in0=dx,
                                in1=bcast_free(basex_sb[:], PXT), op=Alu.add)

        def floor_(src, dst_f, dst_frac, dst_clamp):
            ii = sb.tile([128, PXT, 9], mybir.dt.int32, tag="flo_i")
            nc.vector.tensor_copy(out=ii[:], in_=src[:])
            ff = sb.tile([128, PXT, 9], f32, tag="flo_f")
            nc.vector.tensor_copy(out=ff[:], in_=ii[:])
            gt = sb.tile([128, PXT, 9], f32, tag="flo_g")
            nc.vector.tensor_tensor(out=gt[:], in0=ff[:], in1=src[:],
                                    op=Alu.is_gt)
            nc.vector.tensor_tensor(out=dst_f[:], in0=ff[:], in1=gt[:],
                                    op=Alu.subtract)
            nc.vector.tensor_tensor(out=dst_frac[:], in0=src[:], in1=dst_f[:],
                                    op=Alu.subtract)
            nc.vector.tensor_scalar(out=dst_clamp[:], in0=dst_f[:],
                                    scalar1=0.0, scalar2=65.0,
                                    op0=Alu.max, op1=Alu.min)

        y0f = sb.tile([128, PXT, 9], f32)
        ly = sb.tile([128, PXT, 9], f32)
        y0c = sb.tile([128, PXT, 9], f32)
        floor_(pyg, y0f, ly, y0c)
        x0f = sb.tile([128, PXT, 9], f32)
        lx = sb.tile([128, PXT, 9], f32)
        x0c = sb.tile([128, PXT, 9], f32)
        floor_(pxg, x0f, lx, x0c)

        # ------- phase 3: wrapped int16 gather indices (before coefs, so
        # ------- the first gather launches as early as possible) -----------
        # idxf [128, 9k, 16t] fp32 ; f = k*16 + t
        idxf = sb.tile([128, 9, PXT], f32)
        idx_v = swap_free(idxf[:])            # iterate (t, k)
        tmpi = sb.tile([128, PXT, 9], f32)
        nc.vector.tensor_scalar_mul(out=tmpi[:], in0=y0c[:], scalar1=66.0)
        nc.vector.tensor_tensor(out=idx_v, in0=tmpi[:], in1=x0c[:], op=Alu.add)

        # double transpose: [128p, 144f] -> [f, 128p] -> wrapped [16a, ...]
        idxf_flat = idxf[:].rearrange("p a b -> p (a b)")
        chunks = [(0, 96), (96, 48)]
        t1sb = sb.tile([96, 2, 128], f32)
        for chk, (c0, cw) in enumerate(chunks):
            ps_t1 = ps_small.tile([96, 128], f32, tag="ps")
            nc.tensor.transpose(ps_t1[0:cw, :],
                                idxf_flat[:, c0:c0 + cw],
                                ident32[:, :])
            nc.scalar.copy(out=t1sb[0:cw, chk, :], in_=ps_t1[0:cw, :])

        # indices must be replicated across all 8 partition groups of 16
        # (each Q7 core reads its own group) -> replication matmul with rep16.
        # Separate tiles per chunk so taps 0-5 don't wait on taps 6-8.
        wbufs = [sb.tile([128, 6, PXT, 8], i16, tag="wbuf0", name="wbuf0"),
                 sb.tile([128, 3, PXT, 8], i16, tag="wbuf1", name="wbuf1")]
        for chk, (c0, cw) in enumerate(chunks):
            nfr = cw // 16
            for u in range(8):
                ps_t2 = ps_small.tile([16, 96], f32, tag="ps")
                nc.tensor.transpose(ps_t2[:, 0:cw],
                                    t1sb[0:cw, chk, u * 16:(u + 1) * 16],
                                    ident32[0:cw, 0:cw])
                t2sb = sb.tile([16, 96], f32, tag="t2sb")
                nc.scalar.copy(out=t2sb[:, 0:cw], in_=ps_t2[:, 0:cw])
                ps_rep = ps_small.tile([128, 96], f32, tag="ps")
                nc.tensor.matmul(ps_rep[:, 0:cw], rep16_sb[:, :],
                                 t2sb[:, 0:cw], start=True, stop=True)
                nc.vector.tensor_copy(
                    out=wbufs[chk][:, :, :, u],
                    in_=ps_rep[:, 0:cw].rearrange("p (a t) -> p a t", t=PXT))

        def wbuf_k(k):
            return wbufs[0][:, k, :, :] if k < 6 else wbufs[1][:, k - 6, :, :]

        # ---------------- phase 2b: bilinear corner coefficients -----------
        mss = sb.tile([128, PXT, 9], f32)
        nc.vector.tensor_tensor(out=mss[:], in0=moff,
                                in1=bcast_free(bmrep_sb[:], PXT), op=Alu.add)
        msk = sb.tile([128, PXT, 9], f32)
        nc.scalar.activation(out=msk[:], in_=mss[:], func=Act.Sigmoid)

        ly1 = sb.tile([128, PXT, 9], f32)
        nc.vector.tensor_scalar(out=ly1[:], in0=ly[:], scalar1=-1.0,
                                scalar2=1.0, op0=Alu.mult, op1=Alu.add)
        lx1 = sb.tile([128, PXT, 9], f32)
        nc.vector.tensor_scalar(out=lx1[:], in0=lx[:], scalar1=-1.0,
                                scalar2=1.0, op0=Alu.mult, op1=Alu.add)
        ay0 = sb.tile([128, PXT, 9], f32)
        nc.vector.tensor_tensor(out=ay0[:], in0=ly1[:], in1=msk[:], op=Alu.mult)
        ay1 = sb.tile([128, PXT, 9], f32)
        nc.vector.tensor_tensor(out=ay1[:], in0=ly[:], in1=msk[:], op=Alu.mult)
        c00 = sb.tile([128, PXT, 9], f32)
        c01 = sb.tile([128, PXT, 9], f32)
        c10 = sb.tile([128, PXT, 9], f32)
        c11 = sb.tile([128, PXT, 9], f32)
        nc.vector.tensor_tensor(out=c00[:], in0=ay0[:], in1=lx1[:], op=Alu.mult)
        nc.vector.tensor_tensor(out=c01[:], in0=ay0[:], in1=lx[:], op=Alu.mult)
        nc.vector.tensor_tensor(out=c10[:], in0=ay1[:], in1=lx1[:], op=Alu.mult)
        nc.vector.tensor_tensor(out=c11[:], in0=ay1[:], in1=lx[:], op=Alu.mult)
        # clamping maps x0<=-2 (y0<=-2) pairs onto (border, image 0): the
        # second pair element then reads wrong data -> kill +1-corner coefs
        mxv = sb.tile([128, PXT, 9], f32)
        nc.vector.tensor_scalar(out=mxv[:], in0=x0f[:], scalar1=0.0,
                                scalar2=None, op0=Alu.is_ge)
        myv = sb.tile([128, PXT, 9], f32)
        nc.vector.tensor_scalar(out=myv[:], in0=y0f[:], scalar1=0.0,
                                scalar2=None, op0=Alu.is_ge)
        nc.vector.tensor_tensor(out=c01[:], in0=c01[:], in1=mxv[:], op=Alu.mult)
        nc.vector.tensor_tensor(out=c10[:], in0=c10[:], in1=myv[:], op=Alu.mult)
        nc.vector.tensor_tensor(out=c11[:], in0=c11[:], in1=mxv[:], op=Alu.mult)
        nc.vector.tensor_tensor(out=c11[:], in0=c11[:], in1=myv[:], op=Alu.mult)

        # ---------------- phase 4: main loop -------------------------------
        y_sb = sb.tile([128, 2, 2, 1024], f32)      # [o', oh, half, px]
        s1b = sb.tile([128, 2, 2], f32)
        s2b = sb.tile([128, 2, 2], f32)

        y_ps = [ps_y.tile([128, 1024], f32, tag=f"y_ps{oh}", name=f"y_ps{oh}")
                for oh in range(2)]

        # GPSIMD cannot access PSUM -> rotate copies across Act and DVE
        cp_engines = [nc.scalar.copy, nc.vector.tensor_copy]
        cp_i = 0

        for half in range(2):
            for k in range(KK):
                g = gat.tile([128, 8, 1024], f16, tag="g")
                nc.gpsimd.dma_gather(
                    out_ap=g[:], in_ap=xt_win,
                    idxs_ap=wbuf_k(k)[:, half * 8:(half + 1) * 8, :],
                    num_idxs=1024, num_idxs_reg=1024,
                    elem_size=1024, queue_num=0)
                for t8p in range(4):                 # pairs of px tiles
                    ps_v = ps_vt.tile([128, 512], f32, tag="ps_v")
                    dg = [[diags.tile([128, 128], f16, tag=f"d{j}{cn}",
                                      name=f"d{j}{cn}")
                           for cn in range(4)] for j in range(2)]
                    for j in range(2):
                        t8 = t8p * 2 + j
                        t = half * 8 + t8
                        for cn, cf in enumerate((c00, c01, c10, c11)):
                            nc.vector.tensor_scalar_mul(
                                out=dg[j][cn][:], in0=ident16[:],
                                scalar1=cf[:, t, k:k + 1])
                    for j in range(2):
                        t8 = t8p * 2 + j
                        for hh in range(2):
                            for cn in range(4):
                                nc.tensor.matmul(
                                    ps_v[:, j * 256 + hh * 128:
                                         j * 256 + (hh + 1) * 128],
                                    g[:, t8, cn * 256 + hh * 128:
                                      cn * 256 + (hh + 1) * 128],
                                    dg[j][cn][:],
                                    start=(j == 0 and hh == 0 and cn == 0),
                                    stop=(j == 1 and hh == 1 and cn == 3))
                    valT = vals.tile([128, 512], f16, tag="valT")
                    cp_engines[cp_i % 2](out=valT[:], in_=ps_v[:])
                    cp_i += 1
                    for j in range(2):
                        t8 = t8p * 2 + j
                        for oh in range(2):
                            for cc in range(2):
                                # start/stop are per 2KB PSUM bank (= 4 t8
                                # slices): start clears has_written for the
                                # whole bank, so only the first matmul
                                # touching the bank sets it.
                                nc.tensor.matmul(
                                    y_ps[oh][:, t8 * 128:(t8 + 1) * 128],
                                    w2_sb[:, k, cc, oh, :],
                                    valT[:, j * 256 + cc * 128:
                                         j * 256 + (cc + 1) * 128],
                                    start=(k == 0 and cc == 0 and t8 % 4 == 0),
                                    stop=(k == KK - 1 and cc == 1
                                          and t8 % 4 == 3))
            for oh in range(2):
                sq_scratch = sb.tile([128, 1024], f32, tag="sq")
                nc.scalar.activation(out=y_sb[:, oh, half, :], in_=y_ps[oh][:],
                                     func=Act.Copy,
                                     accum_out=s1b[:, oh, half:half + 1])
                nc.scalar.activation(out=sq_scratch[:], in_=y_ps[oh][:],
                                     func=Act.Square,
                                     accum_out=s2b[:, oh, half:half + 1])

        # ---------------- phase 5: GroupNorm -------------------------------
        s1 = sb.tile([128, 2], f32)
        nc.vector.tensor_tensor(out=s1[:], in0=s1b[:, :, 0], in1=s1b[:, :, 1],
                                op=Alu.add)
        s2 = sb.tile([128, 2], f32)
        nc.vector.tensor_tensor(out=s2[:], in0=s2b[:, :, 0], in1=s2b[:, :, 1],
                                op=Alu.add)
        # fold conv bias b: S1' = S1 + NPX*b ; S2' = S2 + 2 b S1 + NPX b^2
        stk = sb.tile([128, 4], f32)
        q1 = sb.tile([128, 2], f32)
        nc.vector.tensor_tensor(out=q1[:], in0=bvec_sb[:], in1=s1[:],
                                op=Alu.mult)
        nc.vector.scalar_tensor_tensor(out=stk[:, 2:4], in0=q1[:], scalar=2.0,
                                       in1=s2[:], op0=Alu.mult, op1=Alu.add)
        q2 = sb.tile([128, 2], f32)
        nc.vector.tensor_tensor(out=q2[:], in0=bvec_sb[:], in1=bvec_sb[:],
                                op=Alu.mult)
        nc.vector.scalar_tensor_tensor(out=stk[:, 2:4], in0=q2[:],
                                       scalar=float(NPX), in1=stk[:, 2:4],
                                       op0=Alu.mult, op1=Alu.add)
        nc.vector.scalar_tensor_tensor(out=stk[:, 0:2], in0=bvec_sb[:],
                                       scalar=float(NPX), in1=s1[:],
                                       op0=Alu.mult, op1=Alu.add)
        ps_s = ps_small.tile([1, 4], f32, tag="ps")
        nc.tensor.matmul(ps_s[:, :], ones_col[:, :], stk[:, :],
                         start=True, stop=True)
        tot4 = sb.tile([1, 4], f32)
        nc.vector.tensor_copy(out=tot4[:], in_=ps_s[:, :])
        ccs = sb.tile([1, 8], f32)
        nc.vector.memset(ccs[:], 0.0)
        nc.vector.tensor_tensor(out=ccs[:, 0:1], in0=tot4[:, 0:1],
                                in1=tot4[:, 1:2], op=Alu.add)
        nc.vector.tensor_tensor(out=ccs[:, 1:2], in0=tot4[:, 2:3],
                                in1=tot4[:, 3:4], op=Alu.add)

        tot = sb.tile([1, 8], f32)
        if use_collective:
            nc.sync.dma_start(out=cc_in[:], in_=ccs[:])
            nc.gpsimd.collective_compute(
                "AllReduce", Alu.add,
                replica_groups=[[0, 1], [2, 3], [4, 5], [6, 7]],
                ins=[cc_in[:].opt()], outs=[cc_out[:].opt()])
            nc.sync.dma_start(out=tot[:], in_=cc_out[:])
        else:
            nc.vector.tensor_scalar_mul(out=tot[:], in0=ccs[:], scalar1=2.0)

        invN = 1.0 / float(C * H * W)
        mu = sb.tile([1, 1], f32)
        nc.vector.tensor_scalar_mul(out=mu[:], in0=tot[:, 0:1], scalar1=invN)
        mu2 = sb.tile([1, 1], f32)
        nc.vector.tensor_tensor(out=mu2[:], in0=mu[:], in1=mu[:], op=Alu.mult)
        var = sb.tile([1, 1], f32)
        nc.vector.scalar_tensor_tensor(out=var[:], in0=tot[:, 1:2],
                                       scalar=invN, in1=mu2[:],
                                       op0=Alu.mult, op1=Alu.subtract)
        std = sb.tile([1, 1], f32)
        nc.scalar.activation(out=std[:], in_=var[:], func=Act.Sqrt,
                             bias=eps_t[:, 0:1])
        rs = sb.tile([1, 1], f32)
        nc.vector.reciprocal(out=rs[:], in_=std[:])
        mr = sb.tile([1, 2], f32)
        nc.vector.tensor_copy(out=mr[:, 0:1], in_=mu[:])
        nc.vector.tensor_copy(out=mr[:, 1:2], in_=rs[:])
        ps_b = ps_small.tile([128, 2], f32, tag="ps")
        nc.tensor.matmul(ps_b[:, :], ones_row[:, :], mr[:, :],
                         start=True, stop=True)
        mr128 = sb.tile([128, 2], f32)
        nc.vector.tensor_copy(out=mr128[:], in_=ps_b[:, :])
        svec = sb.tile([128, 2], f32)
        nc.vector.tensor_scalar_mul(out=svec[:], in0=gam_sb[:],
                                    scalar1=mr128[:, 1:2])
        tdiff = sb.tile([128, 2], f32)
        nc.vector.tensor_scalar_sub(out=tdiff[:], in0=bvec_sb[:],
                                    scalar1=mr128[:, 0:1])
        b2 = sb.tile([128, 2], f32)
        nc.vector.tensor_tensor(out=b2[:], in0=tdiff[:], in1=svec[:],
                                op=Alu.mult)
        nc.vector.tensor_tensor(out=b2[:], in0=b2[:], in1=bet_sb[:],
                                op=Alu.add)

        for oh in range(2):
            for half in range(2):
                nc.scalar.activation(out=y_sb[:, oh, half, :],
                                     in_=y_sb[:, oh, half, :],
                                     func=Act.Relu,
                                     scale=svec[:, oh:oh + 1],
                                     bias=b2[:, oh:oh + 1])
            nc.sync.dma_start(out=yout[oh * 128:(oh + 1) * 128, :],
                              in_=y_sb[:, oh, :, :])

    nc.compile()
    return nc


# ----------------------------------------------------------------- entry
def kernel(x, w_off, b_off, w, b, gamma, beta):
    from concourse.bass_utils import run_bass_kernel_spmd

    in_maps = prep_per_core(np.asarray(x, np.float32),
                            np.asarray(w_off, np.float32),
                            np.asarray(b_off, np.float32),
                            np.asarray(w, np.float32),
                            np.asarray(b, np.float32),
                            np.asarray(gamma, np.float32),
                            np.asarray(beta, np.float32))
    if "nc" not in _cache:
        _cache["nc"] = build_module(use_collective=True)
    res = run_bass_kernel_spmd(_cache["nc"], in_maps,
                               core_ids=list(range(NCORES)))
    out = np.zeros((B, CO, H, W), np.float32)
    for core in range(NCORES):
        bi, hh = core // 2, core % 2
        out[bi, :, hh * 32:(hh + 1) * 32, :] = (
            res.results[core]["yout"].reshape(CO, 32, 64))
    return out


# revision 26
# speedup vs baseline: 2.3679x; 1.1194x over previous
"""Trainium2 Bass kernel for nn_DCNConvModule (modulated deformable conv
+ GroupNorm(1) + ReLU).

Sharding: 8 cores; core (2b + h) computes sample b, output rows [32h, 32h+32).
GroupNorm statistics are per-sample -> tiny AllReduce of (sum, sumsq) within
core pairs [[0,1],[2,3],[4,5],[6,7]].

Per-core algorithm (pixel-major "px" = 2048 output pixels on 16 tiles of 128),
fp16 hot path:
  1. offset conv as pixel-major PE matmuls -> offT [px, 27] directly
     (lhsT = fp16 x-slab window, rhs = fp16 conv_offset weights).
  2. coef/index math on DVE in fp32; a zero-padded "quad" gather table in
     DRAM (one row = the 4 bilinear corner cells, 1024 fp16) makes corner
     validity masking implicit.
  3. per tap: one dma_gather fetches all 4 corners of 1024 pixels; the
     bilinear+mask combine is fused into PE transposes as diagonal-matrix
     matmuls (diag(coef) built with one 4x-mode tensor_scalar each), with
     the 4 corners accumulated in PSUM; PSUM->SBUF copies rotate across
     Act/DVE/Pool; 4 fp16 matmuls per (tile, tap) accumulate y in PSUM.
  4. GN: per-partition sums via ACT accumulators, ones-matmul partition
     reduce, pair AllReduce, normalize+ReLU as one ACT op per chunk.
"""
import contextlib
import numpy as np

K = 3
KK = 9
C = 256
CO = 256
H = 64
W = 64
B = 4
GW = 66                  # padded grid width
TQ = 4416                # quad-table rows (>= 65*66+65+1; TQ+67 <= 68*66)
PXT = 16                 # 128-pixel tiles per core
NPX = PXT * 128          # 2048 pixels per core
GN_EPS = 1e-5
NCORES = 8

_cache = {}


# ----------------------------------------------------------------- host prep
def prep_per_core(x, w_off, b_off, w, b, gamma, beta):
    """Build the 8 per-core input maps (all numpy, layout-only work)."""
    ki = np.arange(KK) // K
    kj = np.arange(KK) % K

    # conv_offset lhsT  [128, 2, 9, 27]: [c', cc, tap, o]
    woff_r = np.ascontiguousarray(
        w_off.reshape(27, 2, 128, K, K).transpose(2, 1, 3, 4, 0)
        .reshape(128, 2, 9, 27)).astype(np.float16)
    # main DCN lhsT  [128, 9, 2, 2, 128]: [c', k, cc, oh, o']
    w2 = np.ascontiguousarray(
        w.reshape(2, 128, 2, 128, KK).transpose(3, 4, 2, 0, 1)
    ).astype(np.float16)
    bvec = np.ascontiguousarray(b.reshape(2, 128).T).astype(np.float32)
    gam2 = np.ascontiguousarray(gamma.reshape(2, 128).T).astype(np.float32)
    bet2 = np.ascontiguousarray(beta.reshape(2, 128).T).astype(np.float32)

    p = np.arange(128)
    # dy/dx conv biases folded into the base sampling coordinates
    basex = (p[:, None] % 64 + kj[None, :]
             + b_off[2 * np.arange(KK) + 1][None, :]).astype(np.float32)
    # mask conv bias, replicated across partitions
    bmrep = np.broadcast_to(b_off[18:27][None, :], (128, 9)).astype(np.float32)
    bmrep = np.ascontiguousarray(bmrep)

    x_pad = np.zeros((B, C, H + 2, W + 2), np.float32)
    x_pad[:, :, 1:H+1, 1:W+1] = x

    in_maps = []
    for core in range(NCORES):
        bi, hh = core // 2, core % 2
        h0 = hh * 32
        # quad table: row i = cells [i, i+1, i+66, i+67] of the flat padded
        # grid (row-major 66 wide, rows 66-67 zero) -> all 4 bilinear
        # corners of (y0, x0) with implicit zero at every wrap.
        grid = np.zeros((68, GW, C), np.float32)
        grid[:66, :, :] = x_pad[bi].transpose(1, 2, 0)
        flat = grid.reshape(68 * GW, C)
        quad = np.concatenate(
            [flat[0:TQ], flat[1:TQ+1], flat[66:TQ+66], flat[67:TQ+67]],
            axis=1).astype(np.float16)
        slab = x_pad[bi][:, h0:h0+34, :]                    # [256, 34, 66]
        # three column-shifted 64-wide copies: any (row-pair, tap) window of
        # 128 pixels is then contiguous (stationary matmul operands must
        # have a single free dim)
        sl3 = np.stack([slab[:, :, bb:bb+64] for bb in range(3)], axis=1)
        xcf = np.ascontiguousarray(
            sl3.reshape(2, 128, 3, 34, 64).transpose(1, 0, 2, 3, 4)
        ).astype(np.float16)                                # [128,2,3,34,64]
        xca = np.ascontiguousarray(xcf[:, :, :, 0:18].reshape(128, 2, 3, -1))
        xcb = np.ascontiguousarray(xcf[:, :, :, 16:34].reshape(128, 2, 3, -1))
        t = np.arange(PXT)
        basey = np.ascontiguousarray(
            (h0 + (t[None, :, None] * 128 + p[:, None, None]) // 64
             + ki[None, None, :])
            + b_off[2 * np.arange(KK)][None, None, :]).astype(np.float32)
        rep = np.zeros((16, 128), np.float32)
        rep[np.arange(128) % 16, np.arange(128)] = 1.0
        in_maps.append(dict(
            xt=np.ascontiguousarray(quad), xca=xca, xcb=xcb,
            woff=woff_r, w2=w2,
            bvec=bvec, gam2=gam2, bet2=bet2, bmrep=bmrep,
            basey=basey, basex=basex, rep16=rep,
        ))
    return in_maps


# --------------------------------------------------------------- bass kernel
def build_module(use_collective=True):
    import concourse.bacc as bacc
    import concourse.bass as bass
    import concourse.tile as tile
    from concourse import mybir
    from concourse.masks import make_identity

    f32 = mybir.dt.float32
    f16 = mybir.dt.float16
    i16 = mybir.dt.int16
    Alu = mybir.AluOpType
    Act = mybir.ActivationFunctionType

    nc = bacc.Bacc("TRN2", target_bir_lowering=False, debug=False,
                   num_devices=NCORES)

    xt = nc.dram_tensor("xt", [TQ, 1024], f16, kind="ExternalInput")
    xca = nc.dram_tensor("xca", [128, 2, 3, 18 * 64], f16, kind="ExternalInput")
    xcb = nc.dram_tensor("xcb", [128, 2, 3, 18 * 64], f16, kind="ExternalInput")
    woff = nc.dram_tensor("woff", [128, 2, 9, 27], f16, kind="ExternalInput")
    w2 = nc.dram_tensor("w2", [128, 9, 2, 2, 128], f16, kind="ExternalInput")
    bvec = nc.dram_tensor("bvec", [128, 2], f32, kind="ExternalInput")
    gam2 = nc.dram_tensor("gam2", [128, 2], f32, kind="ExternalInput")
    bet2 = nc.dram_tensor("bet2", [128, 2], f32, kind="ExternalInput")
    bmrep = nc.dram_tensor("bmrep", [128, 9], f32, kind="ExternalInput")
    basey = nc.dram_tensor("basey", [128, PXT, 9], f32, kind="ExternalInput")
    basex = nc.dram_tensor("basex", [128, 9], f32, kind="ExternalInput")
    rep16 = nc.dram_tensor("rep16", [16, 128], f32, kind="ExternalInput")
    yout = nc.dram_tensor("yout", [CO, NPX], f16, kind="ExternalOutput")

    cc_in = nc.dram_tensor("cc_in", [1, 8], f32)
    cc_out = nc.dram_tensor("cc_out", [1, 8], f32)

    xt_win = bass.AP(tensor=xt, offset=0, ap=[[1024, TQ], [1, 1024]])

    def swap_free(ap2):
        """Swap the two free dims of a [P, A, B] AP (iteration order only)."""
        return bass.AP(tensor=ap2.tensor, offset=ap2.offset,
                       ap=[ap2.ap[0], ap2.ap[2], ap2.ap[1]])

    def bcast_free(ap1, n):
        """View a [P, F] AP as [P, n, F] with stride-0 broadcast."""
        return bass.AP(tensor=ap1.tensor, offset=ap1.offset,
                       ap=[ap1.ap[0], [0, n], ap1.ap[1]])

    with tile.TileContext(nc) as tc, contextlib.ExitStack() as ctx:
        consts = ctx.enter_context(tc.tile_pool(name="consts", bufs=1))
        sb = ctx.enter_context(tc.tile_pool(name="sb", bufs=1))
        ps_small = ctx.enter_context(
            tc.tile_pool(name="ps_small", bufs=1, space="PSUM"))
        gat = ctx.enter_context(tc.tile_pool(name="gat", bufs=4))
        diags = ctx.enter_context(tc.tile_pool(name="diags", bufs=2))
        vals = ctx.enter_context(tc.tile_pool(name="vals", bufs=3))
        ps_vt = ctx.enter_context(
            tc.tile_pool(name="ps_vt", bufs=3, space="PSUM"))
        ps_y = ctx.enter_context(
            tc.tile_pool(name="ps_y", bufs=1, space="PSUM"))

        ident32 = consts.tile([128, 128], f32)
        make_identity(nc, ident32[:])
        ident16 = consts.tile([128, 128], f16)
        make_identity(nc, ident16[:])
        ones_row = consts.tile([1, 128], f32)
        nc.vector.memset(ones_row[:], 1.0)
        ones_col = consts.tile([128, 1], f32)
        nc.vector.memset(ones_col[:], 1.0)
        eps_t = consts.tile([1, 1], f32)
        nc.vector.memset(eps_t[:], GN_EPS)

        # load order: xc+woff gate phase 1, basey/basex/rep16 gate the index
        # path to the first gather; w2 is not needed until the main matmuls
        xca_sb = consts.tile([128, 2, 3, 18 * 64], f16)
        nc.sync.dma_start(out=xca_sb[:], in_=xca[:])
        woff_sb = consts.tile([128, 2, 9, 27], f16)
        nc.sync.dma_start(out=woff_sb[:], in_=woff[:])
        xcb_sb = consts.tile([128, 2, 3, 18 * 64], f16)
        nc.sync.dma_start(out=xcb_sb[:], in_=xcb[:])
        basey_sb = consts.tile([128, PXT, 9], f32)
        nc.sync.dma_start(out=basey_sb[:], in_=basey[:])
        basex_sb = consts.tile([128, 9], f32)
        nc.sync.dma_start(out=basex_sb[:], in_=basex[:])
        rep16_sb = consts.tile([16, 128], f32)
        nc.sync.dma_start(out=rep16_sb[:], in_=rep16[:])
        bmrep_sb = consts.tile([128, 9], f32)
        nc.sync.dma_start(out=bmrep_sb[:], in_=bmrep[:])
        bvec_sb = consts.tile([128, 2], f32)
        nc.sync.dma_start(out=bvec_sb[:], in_=bvec[:])
        gam_sb = consts.tile([128, 2], f32)
        nc.sync.dma_start(out=gam_sb[:], in_=gam2[:])
        bet_sb = consts.tile([128, 2], f32)
        nc.sync.dma_start(out=bet_sb[:], in_=bet2[:])
        w2_sb = consts.tile([128, 9, 2, 2, 128], f16)
        nc.sync.dma_start(out=w2_sb[:], in_=w2[:])

        # ------------- phase 1: offset conv, pixel-major  offT[px, 27] ------
        offT = sb.tile([128, PXT, 27], f32)
        for t in range(PXT):
            src, rbase = (xca_sb, 0) if t < 8 else (xcb_sb, 16)
            ps_off = ps_vt.tile([128, 27], f32, tag="ps_v")
            n = 0
            for cc in range(2):
                for a in range(3):
                    for bb in range(3):
                        r0 = (2 * t + a - rbase) * 64
                        nc.tensor.matmul(
                            ps_off[:, :],
                            src[:, cc, bb, r0: r0 + 128],
                            woff_sb[:, cc, a * 3 + bb, :],
                            start=(n == 0), stop=(n == 17))
                        n += 1
            nc.scalar.copy(out=offT[:, t, :], in_=ps_off[:, :])

        # ---------------- phase 2a: sampling coordinates -------------------
        dy = offT[:, :, 0:18:2]     # [128, 16, 9] strided views
        dx = offT[:, :, 1:18:2]
        moff = offT[:, :, 18:27]

        pyg = sb.tile([128, PXT, 9], f32)
        nc.vector.tensor_tensor(out=pyg[:], in0=dy, in1=basey_sb[:], op=Alu.add)
        pxg = sb.tile([128, PXT, 9], f32)
        nc.vector.tensor_tensor(out=pxg[:], in0=dx,
                                in1=bcast_free(basex_sb[:], PXT), op=Alu.add)

        def floor_(src, dst_f, dst_frac, dst_clamp):
            ii = sb.tile([128, PXT, 9], mybir.dt.int32, tag="flo_i")
            nc.vector.tensor_copy(out=ii[:], in_=src[:])
            ff = sb.tile([128, PXT, 9], f32, tag="flo_f")
            nc.vector.tensor_copy(out=ff[:], in_=ii[:])
            gt = sb.tile([128, PXT, 9], f32, tag="flo_g")
            nc.vector.tensor_tensor(out=gt[:], in0=ff[:], in1=src[:],
                                    op=Alu.is_gt)
            nc.vector.tensor_tensor(out=dst_f[:], in0=ff[:], in1=gt[:],
                                    op=Alu.subtract)
            nc.vector.tensor_tensor(out=dst_frac[:], in0=src[:], in1=dst_f[:],
                                    op=Alu.subtract)
            nc.vector.tensor_scalar(out=dst_clamp[:], in0=dst_f[:],
                                    scalar1=0.0, scalar2=65.0,
                                    op0=Alu.max, op1=Alu.min)

        y0f = sb.tile([128, PXT, 9], f32)
        ly = sb.tile([128, PXT, 9], f32)
        y0c = sb.tile([128, PXT, 9], f32)
        floor_(pyg, y0f, ly, y0c)
        x0f = sb.tile([128, PXT, 9], f32)
        lx = sb.tile([128, PXT, 9], f32)
        x0c = sb.tile([128, PXT, 9], f32)
        floor_(pxg, x0f, lx, x0c)

        # ------- phase 3: wrapped int16 gather indices (before coefs, so
        # ------- the first gather launches as early as possible) -----------
        # idxf [128, 9k, 16t] fp32 ; f = k*16 + t
        idxf = sb.tile([128, 9, PXT], f32)
        idx_v = swap_free(idxf[:])            # iterate (t, k)
        tmpi = sb.tile([128, PXT, 9], f32)
        nc.vector.tensor_scalar_mul(out=tmpi[:], in0=y0c[:], scalar1=66.0)
        nc.vector.tensor_tensor(out=idx_v, in0=tmpi[:], in1=x0c[:], op=Alu.add)

        # double transpose: [128p, 144f] -> [f, 128p] -> wrapped [16a, ...]
        idxf_flat = idxf[:].rearrange("p a b -> p (a b)")
        chunks = [(0, 96), (96, 48)]
        t1sb = sb.tile([96, 2, 128], f32)
        for chk, (c0, cw) in enumerate(chunks):
            ps_t1 = ps_vt.tile([96, 128], f32, tag="ps_v")
            nc.tensor.transpose(ps_t1[0:cw, :],
                                idxf_flat[:, c0:c0 + cw],
                                ident32[:, :])
            nc.scalar.copy(out=t1sb[0:cw, chk, :], in_=ps_t1[0:cw, :])

        # indices must be replicated across all 8 partition groups of 16
        # (each Q7 core reads its own group) -> replication matmul with rep16.
        # Separate tiles per chunk so taps 0-5 don't wait on taps 6-8.
        wbufs = [sb.tile([128, 6, PXT, 8], i16, tag="wbuf0", name="wbuf0"),
                 sb.tile([128, 3, PXT, 8], i16, tag="wbuf1", name="wbuf1")]
        for chk, (c0, cw) in enumerate(chunks):
            nfr = cw // 16
            for u in range(8):
                ps_t2 = ps_vt.tile([16, 96], f32, tag="ps_v")
                nc.tensor.transpose(ps_t2[:, 0:cw],
                                    t1sb[0:cw, chk, u * 16:(u + 1) * 16],
                                    ident32[0:cw, 0:cw])
                t2sb = sb.tile([16, 96], f32, tag="t2sb")
                nc.scalar.copy(out=t2sb[:, 0:cw], in_=ps_t2[:, 0:cw])
                ps_rep = ps_vt.tile([128, 96], f32, tag="ps_v")
                nc.tensor.matmul(ps_rep[:, 0:cw], rep16_sb[:, :],
                                 t2sb[:, 0:cw], start=True, stop=True)
                nc.vector.tensor_copy(
                    out=wbufs[chk][:, :, :, u],
                    in_=ps_rep[:, 0:cw].rearrange("p (a t) -> p a t", t=PXT))

        def wbuf_k(k):
            return wbufs[0][:, k, :, :] if k < 6 else wbufs[1][:, k - 6, :, :]

        # ---------------- phase 2b: bilinear corner coefficients -----------
        mss = sb.tile([128, PXT, 9], f32)
        nc.vector.tensor_tensor(out=mss[:], in0=moff,
                                in1=bcast_free(bmrep_sb[:], PXT), op=Alu.add)
        msk = sb.tile([128, PXT, 9], f32)
        nc.scalar.activation(out=msk[:], in_=mss[:], func=Act.Sigmoid)

        ly1 = sb.tile([128, PXT, 9], f32)
        nc.vector.tensor_scalar(out=ly1[:], in0=ly[:], scalar1=-1.0,
                                scalar2=1.0, op0=Alu.mult, op1=Alu.add)
        lx1 = sb.tile([128, PXT, 9], f32)
        nc.vector.tensor_scalar(out=lx1[:], in0=lx[:], scalar1=-1.0,
                                scalar2=1.0, op0=Alu.mult, op1=Alu.add)
        ay0 = sb.tile([128, PXT, 9], f32)
        nc.vector.tensor_tensor(out=ay0[:], in0=ly1[:], in1=msk[:], op=Alu.mult)
        ay1 = sb.tile([128, PXT, 9], f32)
        nc.vector.tensor_tensor(out=ay1[:], in0=ly[:], in1=msk[:], op=Alu.mult)
        c00 = sb.tile([128, PXT, 9], f32)
        c01 = sb.tile([128, PXT, 9], f32)
        c10 = sb.tile([128, PXT, 9], f32)
        c11 = sb.tile([128, PXT, 9], f32)
        nc.vector.tensor_tensor(out=c00[:], in0=ay0[:], in1=lx1[:], op=Alu.mult)
        nc.vector.tensor_tensor(out=c01[:], in0=ay0[:], in1=lx[:], op=Alu.mult)
        nc.vector.tensor_tensor(out=c10[:], in0=ay1[:], in1=lx1[:], op=Alu.mult)
        nc.vector.tensor_tensor(out=c11[:], in0=ay1[:], in1=lx[:], op=Alu.mult)
        # clamping maps x0<=-2 (y0<=-2) pairs onto (border, image 0): the
        # second pair element then reads wrong data -> kill +1-corner coefs
        mxv = sb.tile([128, PXT, 9], f32)
        nc.vector.tensor_scalar(out=mxv[:], in0=x0f[:], scalar1=0.0,
                                scalar2=None, op0=Alu.is_ge)
        myv = sb.tile([128, PXT, 9], f32)
        nc.vector.tensor_scalar(out=myv[:], in0=y0f[:], scalar1=0.0,
                                scalar2=None, op0=Alu.is_ge)
        nc.vector.tensor_tensor(out=c01[:], in0=c01[:], in1=mxv[:], op=Alu.mult)
        nc.vector.tensor_tensor(out=c10[:], in0=c10[:], in1=myv[:], op=Alu.mult)
        nc.vector.tensor_tensor(out=c11[:], in0=c11[:], in1=mxv[:], op=Alu.mult)
        nc.vector.tensor_tensor(out=c11[:], in0=c11[:], in1=myv[:], op=Alu.mult)

        # ---------------- phase 4: main loop -------------------------------
        y_sb = sb.tile([128, 2, 2, 1024], f32)      # [o', oh, half, px]
        s1b = sb.tile([128, 2, 2], f32)
        s2b = sb.tile([128, 2, 2], f32)

        y_ps = [ps_y.tile([128, 1024], f32, tag=f"y_ps{oh}", name=f"y_ps{oh}")
                for oh in range(2)]

        # GPSIMD cannot access PSUM -> rotate copies across Act and DVE
        cp_engines = [nc.scalar.copy, nc.vector.tensor_copy]
        cp_i = 0

        for half in range(2):
            for k in range(KK):
                g = gat.tile([128, 8, 1024], f16, tag="g")
                nc.gpsimd.dma_gather(
                    out_ap=g[:], in_ap=xt_win,
                    idxs_ap=wbuf_k(k)[:, half * 8:(half + 1) * 8, :],
                    num_idxs=1024, num_idxs_reg=1024,
                    elem_size=1024, queue_num=0)
                for t8p in range(4):                 # pairs of px tiles
                    ps_v = ps_vt.tile([128, 512], f32, tag="ps_v")
                    dg = [[diags.tile([128, 128], f16, tag=f"d{j}{cn}",
                                      name=f"d{j}{cn}")
                           for cn in range(4)] for j in range(2)]
                    for j in range(2):
                        t8 = t8p * 2 + j
                        t = half * 8 + t8
                        for cn, cf in enumerate((c00, c01, c10, c11)):
                            nc.vector.tensor_scalar_mul(
                                out=dg[j][cn][:], in0=ident16[:],
                                scalar1=cf[:, t, k:k + 1])
                    for j in range(2):
                        t8 = t8p * 2 + j
                        for hh in range(2):
                            for cn in range(4):
                                nc.tensor.matmul(
                                    ps_v[:, j * 256 + hh * 128:
                                         j * 256 + (hh + 1) * 128],
                                    g[:, t8, cn * 256 + hh * 128:
                                      cn * 256 + (hh + 1) * 128],
                                    dg[j][cn][:],
                                    start=(j == 0 and hh == 0 and cn == 0),
                                    stop=(j == 1 and hh == 1 and cn == 3))
                    valT = vals.tile([128, 512], f16, tag="valT")
                    cp_engines[cp_i % 2](out=valT[:], in_=ps_v[:])
                    cp_i += 1
                    for j in range(2):
                        t8 = t8p * 2 + j
                        for oh in range(2):
                            for cc in range(2):
                                # start/stop are per 2KB PSUM bank (= 4 t8
                                # slices): start clears has_written for the
                                # whole bank, so only the first matmul
                                # touching the bank sets it.
                                nc.tensor.matmul(
                                    y_ps[oh][:, t8 * 128:(t8 + 1) * 128],
                                    w2_sb[:, k, cc, oh, :],
                                    valT[:, j * 256 + cc * 128:
                                         j * 256 + (cc + 1) * 128],
                                    start=(k == 0 and cc == 0 and t8 % 4 == 0),
                                    stop=(k == KK - 1 and cc == 1
                                          and t8 % 4 == 3))
            for oh in range(2):
                sq_scratch = sb.tile([128, 1024], f32, tag="sq")
                nc.scalar.activation(out=y_sb[:, oh, half, :], in_=y_ps[oh][:],
                                     func=Act.Copy,
                                     accum_out=s1b[:, oh, half:half + 1])
                nc.scalar.activation(out=sq_scratch[:], in_=y_ps[oh][:],
                                     func=Act.Square,
                                     accum_out=s2b[:, oh, half:half + 1])

        # ---------------- phase 5: GroupNorm -------------------------------
        s1 = sb.tile([128, 2], f32)
        nc.vector.tensor_tensor(out=s1[:], in0=s1b[:, :, 0], in1=s1b[:, :, 1],
                                op=Alu.add)
        s2 = sb.tile([128, 2], f32)
        nc.vector.tensor_tensor(out=s2[:], in0=s2b[:, :, 0], in1=s2b[:, :, 1],
                                op=Alu.add)
        # fold conv bias b: S1' = S1 + NPX*b ; S2' = S2 + 2 b S1 + NPX b^2
        stk = sb.tile([128, 4], f32)
        q1 = sb.tile([128, 2], f32)
        nc.vector.tensor_tensor(out=q1[:], in0=bvec_sb[:], in1=s1[:],
                                op=Alu.mult)
        nc.vector.scalar_tensor_tensor(out=stk[:, 2:4], in0=q1[:], scalar=2.0,
                                       in1=s2[:], op0=Alu.mult, op1=Alu.add)
        q2 = sb.tile([128, 2], f32)
        nc.vector.tensor_tensor(out=q2[:], in0=bvec_sb[:], in1=bvec_sb[:],
                                op=Alu.mult)
        nc.vector.scalar_tensor_tensor(out=stk[:, 2:4], in0=q2[:],
                                       scalar=float(NPX), in1=stk[:, 2:4],
                                       op0=Alu.mult, op1=Alu.add)
        nc.vector.scalar_tensor_tensor(out=stk[:, 0:2], in0=bvec_sb[:],
                                       scalar=float(NPX), in1=s1[:],
                                       op0=Alu.mult, op1=Alu.add)
        ps_s = ps_small.tile([1, 4], f32, tag="ps")
        nc.tensor.matmul(ps_s[:, :], ones_col[:, :], stk[:, :],
                         start=True, stop=True)
        tot4 = sb.tile([1, 4], f32)
        nc.vector.tensor_copy(out=tot4[:], in_=ps_s[:, :])
        ccs = sb.tile([1, 8], f32)
        nc.vector.memset(ccs[:], 0.0)
        nc.vector.tensor_tensor(out=ccs[:, 0:1], in0=tot4[:, 0:1],
                                in1=tot4[:, 1:2], op=Alu.add)
        nc.vector.tensor_tensor(out=ccs[:, 1:2], in0=tot4[:, 2:3],
                                in1=tot4[:, 3:4], op=Alu.add)

        tot = sb.tile([1, 8], f32)
        if use_collective:
            nc.sync.dma_start(out=cc_in[:], in_=ccs[:])
            nc.gpsimd.collective_compute(
                "AllReduce", Alu.add,
                replica_groups=[[0, 1], [2, 3], [4, 5], [6, 7]],
                ins=[cc_in[:].opt()], outs=[cc_out[:].opt()])
            nc.sync.dma_start(out=tot[:], in_=cc_out[:])
        else:
            nc.vector.tensor_scalar_mul(out=tot[:], in0=ccs[:], scalar1=2.0)

        invN = 1.0 / float(C * H * W)
        mu = sb.tile([1, 1], f32)
        nc.vector.tensor_scalar_mul(out=mu[:], in0=tot[:, 0:1], scalar1=invN)
        mu2 = sb.tile([1, 1], f32)
        nc.vector.tensor_tensor(out=mu2[:], in0=mu[:], in1=mu[:], op=Alu.mult)
        var = sb.tile([1, 1], f32)
        nc.vector.scalar_tensor_tensor(out=var[:], in0=tot[:, 1:2],
                                       scalar=invN, in1=mu2[:],
                                       op0=Alu.mult, op1=Alu.subtract)
        std = sb.tile([1, 1], f32)
        nc.scalar.activation(out=std[:], in_=var[:], func=Act.Sqrt,
                             bias=eps_t[:, 0:1])
        rs = sb.tile([1, 1], f32)
        nc.vector.reciprocal(out=rs[:], in_=std[:])
        mr = sb.tile([1, 2], f32)
        nc.vector.tensor_copy(out=mr[:, 0:1], in_=mu[:])
        nc.vector.tensor_copy(out=mr[:, 1:2], in_=rs[:])
        ps_b = ps_small.tile([128, 2], f32, tag="ps")
        nc.tensor.matmul(ps_b[:, :], ones_row[:, :], mr[:, :],
                         start=True, stop=True)
        mr128 = sb.tile([128, 2], f32)
        nc.vector.tensor_copy(out=mr128[:], in_=ps_b[:, :])
        svec = sb.tile([128, 2], f32)
        nc.vector.tensor_scalar_mul(out=svec[:], in0=gam_sb[:],
                                    scalar1=mr128[:, 1:2])
        tdiff = sb.tile([128, 2], f32)
        nc.vector.tensor_scalar_sub(out=tdiff[:], in0=bvec_sb[:],
                                    scalar1=mr128[:, 0:1])
        b2 = sb.tile([128, 2], f32)
        nc.vector.tensor_tensor(out=b2[:], in0=tdiff[:], in1=svec[:],
                                op=Alu.mult)
        nc.vector.tensor_tensor(out=b2[:], in0=b2[:], in1=bet_sb[:],
                                op=Alu.add)

        y16 = sb.tile([128, 2, 2, 1024], f16)
        for oh in range(2):
            for half in range(2):
                nc.scalar.activation(out=y16[:, oh, half, :],
                                     in_=y_sb[:, oh, half, :],
                                     func=Act.Relu,
                                     scale=svec[:, oh:oh + 1],
                                     bias=b2[:, oh:oh + 1])
                nc.sync.dma_start(
                    out=yout[oh * 128:(oh + 1) * 128,
                             half * 1024:(half + 1) * 1024],
                    in_=y16[:, oh, half, :])

    nc.compile()
    return nc


# ----------------------------------------------------------------- entry
def kernel(x, w_off, b_off, w, b, gamma, beta):
    from concourse.bass_utils import run_bass_kernel_spmd

    in_maps = prep_per_core(np.asarray(x, np.float32),
                            np.asarray(w_off, np.float32),
                            np.asarray(b_off, np.float32),
                            np.asarray(w, np.float32),
                            np.asarray(b, np.float32),
                            np.asarray(gamma, np.float32),
                            np.asarray(beta, np.float32))
    if "nc" not in _cache:
        _cache["nc"] = build_module(use_collective=True)
    res = run_bass_kernel_spmd(_cache["nc"], in_maps,
                               core_ids=list(range(NCORES)))
    out = np.zeros((B, CO, H, W), np.float32)
    for core in range(NCORES):
        bi, hh = core // 2, core % 2
        out[bi, :, hh * 32:(hh + 1) * 32, :] = (
            res.results[core]["yout"].reshape(CO, 32, 64))
    return out
